# revision 1
# baseline (speedup 1.0000x reference)
"""Complex-valued multi-head attention on 8 Trainium2 NeuronCores.

Sharding: batch(2) x head-pairs(4) -> 8 cores; each core runs one batch
element and 2 heads end-to-end (QKV proj -> complex scores -> |s| softmax
-> AV -> partial W_O), host sums the W_O partials over the 4 cores of each
batch element (tensor-parallel reduce) and transposes to the output layout.

Inputs, weights and all matmuls run in fp16 (psum accumulation is fp32);
projections emit per-head stacked [re;im] tensors so each complex score
matrix is a single K=128 matmul (host packs the stacked weights). Softmax
uses exp(|s|/8 - 1.5) without max-subtraction (|s| is bounded small), the
rowsum rides a ones-column matmul on the transposed probabilities, and the
1/rowsum normalization is applied to the 64-wide AV output instead of the
2048-wide attention matrix. sqrt and exp are batched per q-chunk with
explicit ACT table-set loads (sqrt/exp live in different table sets), and
AV/normalize/W_O for q-chunk i are emitted during q-chunk i+1 so PE never
stalls on the softmax chain at chunk boundaries.
"""
import sys

sys.path.insert(0, "/opt/trn_rl_repo")

import numpy as np

B, NQ, NK, R = 2, 2048, 2048, 512
H, DK, DV = 8, 64, 64
NCORES = 8
NCC = 8          # n-chunks for projection streaming (2048/256)
NCW = 256        # projection n-chunk width
QC = 4           # q-chunks in attention (2048/512)
QCW = 512
KT = 16          # k-tiles (2048/128)

_CACHE = {}
DEBUG = False


def _build_nc():
    import concourse.bass as bass
    import concourse.tile as tile
    from concourse.tile import add_dep_helper
    from concourse import bacc, mybir

    f32 = mybir.dt.float32
    f32r = mybir.dt.float32r
    f16 = mybir.dt.float16
    ALU = mybir.AluOpType
    AF = mybir.ActivationFunctionType

    nc = bacc.Bacc("TRN2", target_bir_lowering=False, debug=False,
                   num_devices=NCORES)

    xpack_e = nc.dram_tensor("xpack", [NCC, 24, 128, NCW], f16,
                             kind="ExternalInput")
    wpack_e = nc.dram_tensor("wpack", [128, 64 * 128], f16,
                             kind="ExternalInput")
    wopack_e = nc.dram_tensor("wopack", [128, 3 * 512], f16,
                              kind="ExternalInput")
    onesr_e = nc.dram_tensor("onesr", [1, 128], f32r, kind="ExternalInput")
    ident_e = nc.dram_tensor("ident", [128, 128], f16, kind="ExternalInput")
    dbg_es = {}
    if DEBUG:
        for nm, w in (("d_v16_h0", NK), ("d_oT_re", NQ),
                      ("d_p00", 1024), ("d_rs0", 512)):
            dbg_es[nm] = nc.dram_tensor(nm, [128, w], f32,
                                        kind="ExternalOutput")
    ore_e = nc.dram_tensor("out_re", [512, NQ], f32, kind="ExternalOutput")
    oim_e = nc.dram_tensor("out_im", [512, NQ], f32, kind="ExternalOutput")

    with tile.TileContext(nc) as tc:
      with nc.allow_low_precision(reason="fp16 softmax path"):
        with tc.tile_pool(name="pers", bufs=1) as pers, \
             tc.tile_pool(name="work", bufs=2) as work, \
             tc.tile_pool(name="pwork", bufs=3) as pwork, \
             tc.tile_pool(name="psA", bufs=1, space="PSUM") as psA:

            # Preload the one ACT table set that covers square/ln/exp/copy
            # so the table-load pass never needs to thrash sets.
            from concourse.hw_specs import get_activation_tables
            tables = list(get_activation_tables(nc.m.arch).keys())
            SQRT_SET = tables.index("sqrt_and_others")
            EXP_SET = tables.index("exp_and_others")

            def load_act_set(set_id):
                lafs = mybir.InstLoadActFuncSet(
                    name=nc.get_next_instruction_name(), ins=[], outs=[],
                    act_func_set_id=set_id)
                lafs.engine = mybir.EngineType.Activation
                nc.scalar.add_instruction(lafs)

            load_act_set(EXP_SET)

            # ---- constants ----
            wp = pers.tile([128, 64 * 128], f16, tag="wp")
            nc.sync.dma_start(wp[:], wpack_e[:])
            wop = pers.tile([128, 3 * 512], f16, tag="wop")
            nc.sync.dma_start(wop[:], wopack_e[:])
            ones_row = pers.tile([1, 128], f32r, tag="ones_row")
            nc.sync.dma_start(ones_row[:], onesr_e[:])
            ident16 = pers.tile([128, 128], f16, tag="ident16")
            nc.sync.dma_start(ident16[:], ident_e[:])
            ones16 = pers.tile([128, 1], f16, tag="ones16")
            nc.vector.memset(ones16[:], 1.0)
            eb_exp = pers.tile([128, 1], f32, tag="eb_exp")
            nc.vector.memset(eb_exp[:], -1.5)          # exp(mag - 1.5)

            # ---- projections -> per-head stacked [re;im] tensors fp16 ----
            # q_sb[h] = [q_r_h; q_i_h], A_sb[h] = [k_r_h; k_i_h],
            # C_sb[h] = [-k_i_h; k_r_h], vT16_h[h] = [v_r_h; v_i_h]
            q_sb = [pers.tile([128, NQ], f16, tag=f"q_sb{h}",
                              name=f"q_sb{h}") for h in (0, 1)]
            A_sb = [pers.tile([128, NK], f16, tag=f"A_sb{h}",
                              name=f"A_sb{h}") for h in (0, 1)]
            C_sb = [pers.tile([128, NK], f16, tag=f"C_sb{h}",
                              name=f"C_sb{h}") for h in (0, 1)]
            vT16_h = [pers.tile([128, NK], f16, tag=f"vT16_h{h}",
                                name=f"vT16_h{h}") for h in (0, 1)]

            # dest i uses weight blocks w=2i (for x_re) and w=2i+1 (for x_im)
            # x-block index: t=0..1 q_re/q_im, 2..3 k, 4..5 v
            specs = [
                (q_sb[0], 0), (q_sb[1], 0), (A_sb[0], 2), (A_sb[1], 2),
                (C_sb[0], 2), (C_sb[1], 2), (vT16_h[0], 4), (vT16_h[1], 4),
            ]
            v16_h = [pers.tile([128, NK], f16, tag=f"v16_h{h}",
                               name=f"v16_h{h}") for h in (0, 1)]
            for ncc in range(NCC):
                xt = work.tile([128, 24 * NCW], f16, tag="xt")
                nc.sync.dma_start(
                    xt[:].rearrange("p (b f) -> p b f", f=NCW),
                    xpack_e[ncc].rearrange("b p f -> p b f"))

                def xblk(t, rc):
                    return xt[:, (t * 4 + rc) * NCW:(t * 4 + rc + 1) * NCW]

                def wblk(w, rc):
                    return wp[:, (w * 4 + rc) * 128:(w * 4 + rc + 1) * 128]

                for si, (dest, tx) in enumerate(specs):
                    pj = psA.tile([128, NCW], f32,
                                  tag=("s_re" if si % 2 == 0 else "s_im"),
                                  name=f"pj_{ncc}_{si}")
                    for rc in range(4):
                        nc.tensor.matmul(pj[:], wblk(2 * si, rc),
                                         xblk(tx, rc),
                                         start=(rc == 0), stop=False)
                    for rc in range(4):
                        nc.tensor.matmul(pj[:], wblk(2 * si + 1, rc),
                                         xblk(tx + 1, rc),
                                         start=False, stop=(rc == 3))
                    cs = slice(ncc * NCW, (ncc + 1) * NCW)
                    if si % 2 == 0:
                        nc.scalar.copy(dest[:, cs], pj[:])
                    else:
                        nc.vector.tensor_copy(dest[:, cs], pj[:])
                # transpose this ncc's V columns right away
                for h in (0, 1):
                    for nt in (2 * ncc, 2 * ncc + 1):
                        blk = slice(nt * 128, (nt + 1) * 128)
                        vt_ps = psA.tile([128, 128], f16, tag="s_im",
                                         name=f"vtp_{h}_{nt}")
                        nc.tensor.transpose(vt_ps[:], vT16_h[h][:, blk],
                                            ident16[:])
                        if (h + nt) % 2 == 0:
                            nc.vector.tensor_copy(v16_h[h][:, blk], vt_ps[:])
                        else:
                            nc.scalar.copy(v16_h[h][:, blk], vt_ps[:])

            # ---- output accumulators for W_O ----
            oT_re = pers.tile([128, NQ], f16, tag="oT_re")
            oT_im = pers.tile([128, NQ], f16, tag="oT_im")

            # ---- attention ----
            def emit_wo(qs, qcn):
                for Rc in range(4):
                    wo_re = psA.tile([128, QCW], f32, tag="s_re",
                                     name=f"wore_{Rc}_{qcn}")
                    wo_im = psA.tile([128, QCW], f32, tag="s_im",
                                     name=f"woim_{Rc}_{qcn}")

                    def wob(w):
                        return wop[:, w * 512 + Rc * 128:
                                   w * 512 + Rc * 128 + 128]

                    nc.tensor.matmul(wo_re[:], wob(0), oT_re[:, qs],
                                     start=True, stop=False)
                    nc.tensor.matmul(wo_re[:], wob(2), oT_im[:, qs],
                                     start=False, stop=True)
                    nc.tensor.matmul(wo_im[:], wob(1), oT_re[:, qs],
                                     start=True, stop=False)
                    nc.tensor.matmul(wo_im[:], wob(0), oT_im[:, qs],
                                     start=False, stop=True)
                    st_re = work.tile([128, QCW], f32, tag="st_re")
                    nc.vector.tensor_copy(st_re[:], wo_re[:])
                    nc.sync.dma_start(
                        ore_e[Rc * 128:(Rc + 1) * 128, qs], st_re[:])
                    st_im = work.tile([128, QCW], f32, tag="st_im")
                    nc.vector.tensor_copy(st_im[:], wo_im[:])
                    nc.sync.dma_start(
                        oim_e[Rc * 128:(Rc + 1) * 128, qs], st_im[:])

            def emit_norm(o_ps, rs_ps, qs, qcn):
                bc = psA.tile([128, 1024], f32, tag="s_re",
                              name=f"bc_{qcn}")
                for h in (0, 1):
                    recip = work.tile([1, QCW], f32r, tag=f"recip{h}")
                    nc.vector.reciprocal(recip[:], rs_ps[h][0:1, :])
                    nc.tensor.matmul(bc[:, h * 512:h * 512 + 512],
                                     ones_row[:], recip[:],
                                     start=True, stop=True)
                bc_sb = work.tile([128, 1024], f32r, tag="bc_sb")
                nc.scalar.copy(bc_sb[:], bc[:])
                for h in (0, 1):
                    for ri, dest in ((0, oT_re), (1, oT_im)):
                        rows = slice(64 * ri, 64 * ri + 64)
                        nc.vector.scalar_tensor_tensor(
                            dest[64 * h:64 * h + 64, qs],
                            o_ps[h][rows, :], 1.0,
                            bc_sb[rows, h * 512:h * 512 + 512],
                            ALU.mult, ALU.mult)

            NB = 2  # transcendental batches per q-chunk

            def emit_av(pbs, qs, qcn):
                o_ps = [psA.tile([128, QCW], f32, tag=f"o{h}",
                                 name=f"o{h}_{qcn}") for h in (0, 1)]
                rs_ps = [psA.tile([128, QCW], f32, tag=f"rs{h}",
                                  name=f"rs{h}_{qcn}") for h in (0, 1)]
                HB = KT // NB
                for half in range(NB):
                    for k8 in range(HB):
                        kt = half * HB + k8
                        for h in (0, 1):
                            col = slice(k8 * 1024 + h * 512,
                                        k8 * 1024 + h * 512 + 512)
                            vblk = v16_h[h][:, kt * 128:(kt + 1) * 128]
                            nc.tensor.matmul(o_ps[h][:, :], vblk,
                                             pbs[half][:, col],
                                             start=(kt == 0),
                                             stop=(kt == KT - 1))
                            nc.tensor.matmul(rs_ps[h][0:1, :], ones16[:],
                                             pbs[half][:, col],
                                             start=(kt == 0),
                                             stop=(kt == KT - 1))
                return o_ps, rs_ps

            pending = None
            for qc in range(QC):
                qs = slice(qc * QCW, (qc + 1) * QCW)
                HB = KT // NB  # k-tiles per batch
                bts = []
                pbs = []
                for half in range(NB):
                    bt = work.tile([128, HB * 1024], f16, tag="batch",
                                   bufs=4, name=f"ssqb_{qc}_{half}")
                    bts.append(bt)
                    for k8 in range(HB):
                        kt = half * HB + k8
                        ks = slice(kt * 128, (kt + 1) * 128)
                        s_re = psA.tile([128, 1024], f32, tag="s_re",
                                        name=f"s_re_{qc}_{kt}")
                        s_im = psA.tile([128, 1024], f32, tag="s_im",
                                        name=f"s_im_{qc}_{kt}")
                        for h in (0, 1):
                            col = slice(h * 512, h * 512 + 512)
                            nc.tensor.matmul(s_re[:, col], A_sb[h][:, ks],
                                             q_sb[h][:, qs],
                                             start=True, stop=True)
                            nc.tensor.matmul(s_im[:, col], C_sb[h][:, ks],
                                             q_sb[h][:, qs],
                                             start=True, stop=True)
                        t16 = work.tile([128, 1024], f16, tag="t16")
                        nc.vector.tensor_copy(t16[:], s_re[:])
                        sqre = work.tile([128, 1024], f16, tag="sqre")
                        nc.vector.tensor_mul(sqre[:], t16[:], t16[:])
                        sqim = work.tile([128, 1024], f16, tag="sqim")
                        if kt % 3 == 1:
                            ti16 = work.tile([128, 1024], f16, tag="ti16")
                            nc.vector.tensor_copy(ti16[:], s_im[:])
                            nc.vector.tensor_mul(sqim[:], ti16[:], ti16[:])
                        else:
                            nc.scalar.square(sqim[:], s_im[:])
                        nc.gpsimd.tensor_tensor(
                            bt[:, k8 * 1024:(k8 + 1) * 1024],
                            sqre[:], sqim[:], ALU.add)
                    # sqrt of this half right away (Square stays legal in
                    # sqrt_and_others, so no extra table traffic)
                    if half == 0:
                        load_act_set(SQRT_SET)
                    pb = work.tile([128, HB * 1024], f16, tag="batch",
                                   bufs=4, name=f"pb_{qc}_{half}")
                    pbs.append(pb)
                    nc.scalar.activation(pb[:], bt[:], AF.Sqrt,
                                         scale=1.0 / 64.0)
                load_act_set(EXP_SET)
                for pb in pbs:
                    nc.scalar.activation(pb[:], pb[:], AF.Exp,
                                         bias=eb_exp[:])
                if DEBUG and qc == 0:
                    dp = pers.tile([128, 1024], f32, tag="dbg_p00",
                                   name="dbg_p00")
                    nc.vector.tensor_copy(dp[:], pbs[0][:, 0:1024])
                    nc.sync.dma_start(dbg_es["d_p00"][:], dp[:])
                if pending is not None:
                    ppbs, pqs, pqc = pending
                    o_ps, rs_ps = emit_av(ppbs, pqs, pqc)
                    emit_norm(o_ps, rs_ps, pqs, pqc)
                    emit_wo(pqs, pqc)
                pending = (pbs, qs, qc)
            ppbs, pqs, pqc = pending
            o_ps, rs_ps = emit_av(ppbs, pqs, pqc)
            emit_norm(o_ps, rs_ps, pqs, pqc)
            emit_wo(pqs, pqc)

            if DEBUG:
                for nm, t in (("d_qT_re", qT_re), ("d_kT_re", kT_re),
                              ("d_kT_imn", kT_imn), ("d_vT16_h0", vT16_h[0]),
                              ("d_v16_h0", v16_h[0]), ("d_oT_re", oT_re)):
                    dd = pers.tile(list(t.shape), f32, tag=f"dbg_{nm}",
                                   name=f"dbg_{nm}")
                    nc.vector.tensor_copy(dd[:], t[:])
                    nc.sync.dma_start(dbg_es[nm][:], dd[:])

    nc.finalize()
    return nc


def _get_nc():
    if "nc" not in _CACHE:
        _CACHE["nc"] = _build_nc()
    return _CACHE["nc"]


def _core_inputs(c, inputs):
    b = c // 4
    h0 = 2 * (c % 4)
    hs = slice(h0 * 64, h0 * 64 + 128)

    xpack = np.empty((NCC, 24, 128, NCW), np.float16)
    for t, name in enumerate(
            ("Q_real", "Q_imag", "K_real", "K_imag", "V_real", "V_imag")):
        xT = np.ascontiguousarray(inputs[name][b].T)          # (512, 2048)
        xpack[:, t * 4:(t + 1) * 4] = (
            xT.reshape(4, 128, NCC, NCW).transpose(2, 0, 1, 3))

    wlist = []
    for kind in ("q", "A", "C", "v"):
        base_r = inputs[{"q": "wq_r", "A": "wk_r", "C": "wk_r",
                         "v": "wv_r"}[kind]]
        base_i = inputs[{"q": "wq_i", "A": "wk_i", "C": "wk_i",
                         "v": "wv_i"}[kind]]
        for hh in (0, 1):
            rows = slice((h0 + hh) * 64, (h0 + hh) * 64 + 64)
            wr, wi_ = base_r[rows], base_i[rows]
            if kind == "C":
                # rows = [-k_i; k_r]
                w1 = np.vstack([-wi_, wr])       # x_re weights
                w2 = np.vstack([-wr, -wi_])      # x_im weights
            else:
                # rows = [p_r; p_i]
                w1 = np.vstack([wr, wi_])
                w2 = np.vstack([-wi_, wr])
            wlist += [w1, w2]
    arr = np.empty((64, 128, 128), np.float16)
    for wi, mat in enumerate(wlist):
        arr[wi * 4:(wi + 1) * 4] = np.ascontiguousarray(mat.T).reshape(
            4, 128, 128)
    wpack = np.ascontiguousarray(arr.transpose(1, 0, 2)).reshape(
        128, 64 * 128)

    wo_r_T = np.ascontiguousarray(inputs["wo_r"][:, hs].T)    # (128, 512)
    wo_i_T = np.ascontiguousarray(inputs["wo_i"][:, hs].T)
    wopack = np.concatenate([wo_r_T, wo_i_T, -wo_i_T], axis=1)
    wopack = np.ascontiguousarray(wopack).astype(np.float16)

    return {
        "xpack": xpack,
        "wpack": wpack,
        "wopack": wopack,
        "onesr": np.ones((1, 128), np.float32),
        "ident": np.eye(128, dtype=np.float16),
    }


def kernel(**inputs):
    from concourse.bass_utils import run_bass_kernel_spmd

    nc = _get_nc()
    in_maps = [_core_inputs(c, inputs) for c in range(NCORES)]
    res = run_bass_kernel_spmd(nc, in_maps, list(range(NCORES)))
    out = np.empty((B, NQ, R, 2), np.float32)
    for b in range(B):
        re = np.zeros((512, NQ), np.float64)
        im = np.zeros((512, NQ), np.float64)
        for c in range(b * 4, b * 4 + 4):
            re += res.results[c]["out_re"]
            im += res.results[c]["out_im"]
        out[b, :, :, 0] = re.T
        out[b, :, :, 1] = im.T
    return out



# revision 22
# speedup vs baseline: 1.1442x; 1.1442x over previous
"""Complex-valued multi-head attention on 8 Trainium2 NeuronCores.

Sharding: batch(2) x head-pairs(4) -> 8 cores; each core runs one batch
element and 2 heads end-to-end (QKV proj -> complex scores -> |s| softmax
-> AV -> partial W_O), host sums the W_O partials over the 4 cores of each
batch element (tensor-parallel reduce) and transposes to the output layout.

v2 restructure vs baseline:
- score tiles are [128 kpos, 512 re | 512 im] per (ktile, head); one fused
  square pass (DVE tensor_tensor mult or ACT Square, mixed by ratio) reads
  the f32 PSUM pair and emits re^2|im^2 f16, replacing the old
  copy+mul+square chain.
- z = re^2+im^2 adds run on the otherwise-idle Pool engine (mixed with
  some DVE), rowsum accumulation also on Pool with a single PE ones-fold
  per q-chunk, freeing ~27us of PE rowsum matmuls.
- s_im uses q2 = [q_i; -q_r] derived by two cheap copies per n-chunk
  instead of a second projected K tensor (C_sb), dropping 128 projection
  matmuls.
- ACT table set switches 2x per q-chunk (9 loads total vs 25).
- projections for n-chunks 4..7 are emitted between the first q-chunk's
  halves so PE never idles during the attention warm-up.
"""
import sys

sys.path.insert(0, "/opt/trn_rl_repo")

import numpy as np

B, NQ, NK, R = 2, 2048, 2048, 512
H, DK, DV = 8, 64, 64
NCORES = 8
NCC = 8          # n-chunks for projection streaming (2048/256)
NCW = 256        # projection n-chunk width
QC = 4           # q-chunks in attention (2048/512)
QCW = 512
KT = 16          # k-tiles (2048/128)
HKT = 8          # k-tiles per half-batch

# engine-mix tuning. HW allows only ONE non-scalar PSUM operand per
# vector op, so the score extraction is either a single fused ACT Square
# (unary, one psum read) or a DVE copy + sbuf squares. ACT-fused tiles
# sit at the END of each half-stream where ACT is idle (at the start it
# is still finishing the previous half's sqrt/exp batches).
ACT_FUSED_FROM = 10   # tile_in_half >= this -> fused ACT square
ZADD_DVE_NUM = 0      # out of ZADD_MOD adds go to DVE
ZADD_MOD = 8
ST_ACT_NUM = 0        # out of ST_MOD W_O output copies go to ACT
ST_MOD = 2
PRJ_ACT = False       # projection copies on ACT
VT_ACT = True         # v-transpose copies on ACT
Q2_ACT = False        # q2 derivation on ACT

_CACHE = {}


def _build_nc():
    import concourse.bass as bass
    import concourse.tile as tile
    from concourse import bacc, mybir

    f32 = mybir.dt.float32
    f32r = mybir.dt.float32r
    f16 = mybir.dt.float16
    ALU = mybir.AluOpType
    AF = mybir.ActivationFunctionType

    nc = bacc.Bacc("TRN2", target_bir_lowering=False, debug=False,
                   num_devices=NCORES)

    xpack_e = nc.dram_tensor("xpack", [NCC, 24, 128, NCW], f16,
                             kind="ExternalInput")
    wpack_e = nc.dram_tensor("wpack", [128, 48 * 128], f16,
                             kind="ExternalInput")
    wopack_e = nc.dram_tensor("wopack", [128, 3 * 512], f16,
                              kind="ExternalInput")
    onesr_e = nc.dram_tensor("onesr", [1, 128], f32r, kind="ExternalInput")
    ident_e = nc.dram_tensor("ident", [128, 128], f16, kind="ExternalInput")
    ore_e = nc.dram_tensor("out_re", [512, NQ], f32, kind="ExternalOutput")
    oim_e = nc.dram_tensor("out_im", [512, NQ], f32, kind="ExternalOutput")

    with tile.TileContext(nc) as tc:
      with nc.allow_low_precision(reason="fp16 softmax path"):
        with tc.tile_pool(name="pers", bufs=1) as pers, \
             tc.tile_pool(name="work", bufs=2) as work, \
             tc.tile_pool(name="psA", bufs=1, space="PSUM") as psA:

            # act-table loads are auto-inserted by Bacc.insert_act_table_loads
            # on the scheduled order; manual loads just get hoisted uselessly.

            # ---- constants ----
            wp = pers.tile([128, 48 * 128], f16, tag="wp")
            nc.sync.dma_start(wp[:], wpack_e[:])
            wop = pers.tile([128, 3 * 512], f16, tag="wop")
            nc.sync.dma_start(wop[:], wopack_e[:])
            ones_row = pers.tile([1, 128], f32r, tag="ones_row")
            nc.sync.dma_start(ones_row[:], onesr_e[:])
            ident16 = pers.tile([128, 128], f16, tag="ident16")
            nc.sync.dma_start(ident16[:], ident_e[:])
            ones16 = pers.tile([128, 1], f16, tag="ones16")
            nc.vector.memset(ones16[:], 1.0)
            eb_exp = pers.tile([128, 1], f32, tag="eb_exp")
            nc.vector.memset(eb_exp[:], -1.5)          # exp(mag - 1.5)

            # ---- projection destinations (h-major: cols h*2048 + n) ----
            q_all = pers.tile([128, 2 * NQ], f16, tag="q_all")
            q2_all = pers.tile([128, 2 * NQ], f16, tag="q2_all")
            A_all = pers.tile([128, 2 * NK], f16, tag="A_all")
            vT_all = pers.tile([128, 2 * NK], f16, tag="vT_all")
            v16_all = pers.tile([128, 2 * NK], f16, tag="v16_all")
            oT_re = pers.tile([128, NQ], f16, tag="oT_re")
            oT_im = pers.tile([128, NQ], f16, tag="oT_im")

            # spec s uses weight blocks 2s (x_re) and 2s+1 (x_im);
            # x-block index: t=0..1 q_re/q_im, 2..3 k, 4..5 v
            grp_dest = [q_all, A_all, vT_all]

            cnt = {"ext": 0, "zadd": 0, "prcp": 0, "stc": 0}

            def proj(ncc):
                xt = work.tile([128, 24 * NCW], f16, tag="xt",
                               name=f"xt_{ncc}")
                nc.sync.dma_start(
                    xt[:].rearrange("p (b f) -> p b f", f=NCW),
                    xpack_e[ncc].rearrange("b p f -> p b f"))

                def xblk(t, rc):
                    return xt[:, (t * 4 + rc) * NCW:(t * 4 + rc + 1) * NCW]

                def wblk(w, rc):
                    return wp[:, (w * 4 + rc) * 128:(w * 4 + rc + 1) * 128]

                cs0 = ncc * NCW
                for grp in range(3):
                    pj = psA.tile([128, 512], f32, tag="sb", bufs=2,
                                  name=f"pj_{ncc}_{grp}")
                    for sub in range(2):
                        s = grp * 2 + sub
                        tx = grp * 2
                        dst = pj[:, sub * 256:(sub + 1) * 256]
                        for rc in range(4):
                            nc.tensor.matmul(dst, wblk(2 * s, rc),
                                             xblk(tx, rc),
                                             start=(rc == 0), stop=False)
                        for rc in range(4):
                            nc.tensor.matmul(dst, wblk(2 * s + 1, rc),
                                             xblk(tx + 1, rc),
                                             start=False, stop=(rc == 3))
                    dest = grp_dest[grp]
                    dap = dest[:].rearrange("p (h n) -> p h n", h=2)[
                        :, :, cs0:cs0 + NCW]
                    pap = pj[:].rearrange("p (h n) -> p h n", h=2)
                    if PRJ_ACT:
                        nc.scalar.copy(dap, pap)
                    else:
                        nc.vector.tensor_copy(dap, pap)
                    cnt["prcp"] += 1
                # q2 = [q_i; -q_r] for this chunk (both heads)
                q2ap_t = q2_all[0:64].rearrange("p (h n) -> p h n", h=2)[
                    :, :, cs0:cs0 + NCW]
                qap_b = q_all[64:128].rearrange("p (h n) -> p h n", h=2)[
                    :, :, cs0:cs0 + NCW]
                if Q2_ACT:
                    nc.scalar.copy(q2ap_t, qap_b)
                else:
                    nc.vector.tensor_scalar_mul(q2ap_t, qap_b, 1.0)
                q2ap_b = q2_all[64:128].rearrange("p (h n) -> p h n", h=2)[
                    :, :, cs0:cs0 + NCW]
                qap_t = q_all[0:64].rearrange("p (h n) -> p h n", h=2)[
                    :, :, cs0:cs0 + NCW]
                if Q2_ACT:
                    nc.scalar.mul(q2ap_b, qap_t, -1.0)
                else:
                    nc.vector.tensor_scalar_mul(q2ap_b, qap_t, -1.0)
                # transpose this chunk's V columns
                for h in (0, 1):
                    for nt in (2 * ncc, 2 * ncc + 1):
                        blk = slice(h * NK + nt * 128,
                                    h * NK + (nt + 1) * 128)
                        vt_ps = psA.tile([128, 128], f16, tag=f"o{h}",
                                         name=f"vtp_{h}_{nt}")
                        nc.tensor.transpose(vt_ps[:], vT_all[:, blk],
                                            ident16[:])
                        if VT_ACT:
                            nc.scalar.copy(v16_all[:, blk], vt_ps[:])
                        else:
                            nc.vector.tensor_copy(v16_all[:, blk], vt_ps[:])

            # ---- attention helpers ----
            def scores_tile(qc, kt, h, bt):
                qs0 = qc * QCW
                ks = slice(h * NK + kt * 128, h * NK + (kt + 1) * 128)
                qsl = slice(h * NQ + qs0, h * NQ + qs0 + QCW)
                sb = psA.tile([128, 1024], f32, tag="sb", bufs=2,
                              name=f"sb_{qc}_{kt}_{h}")
                nc.tensor.matmul(sb[:, 0:512], A_all[:, ks],
                                 q_all[:, qsl], start=True, stop=True)
                nc.tensor.matmul(sb[:, 512:1024], A_all[:, ks],
                                 q2_all[:, qsl], start=True, stop=True)
                # square extraction: sq = sb*sb (re^2 | im^2), f16
                sq = work.tile([128, 1024], f16, tag="sq", bufs=4,
                               name=f"sq_{qc}_{kt}_{h}")
                tile_in_half = (kt % HKT) * 2 + h
                if tile_in_half >= ACT_FUSED_FROM:
                    # unary ACT square reads psum once -- legal and fused
                    nc.scalar.square(sq[:], sb[:])
                else:
                    # DVE path: one psum->sbuf copy, then sbuf squares
                    t16 = work.tile([128, 1024], f16, tag="t16", bufs=4,
                                    name=f"t16_{qc}_{kt}_{h}")
                    nc.vector.tensor_copy(t16[:], sb[:])
                    nc.vector.tensor_tensor(sq[:, 0:512], t16[:, 0:512],
                                            t16[:, 0:512], ALU.mult)
                    nc.gpsimd.tensor_tensor(sq[:, 512:1024],
                                            t16[:, 512:1024],
                                            t16[:, 512:1024], ALU.mult)
                cnt["ext"] += 1
                # z = re^2 + im^2 -> bt column slot
                zdst = bt[:, kt * 1024 + h * 512:kt * 1024 + h * 512 + 512]
                if ZADD_DVE_NUM and cnt["zadd"] % ZADD_MOD < ZADD_DVE_NUM:
                    nc.vector.tensor_tensor(zdst, sq[:, 0:512],
                                            sq[:, 512:1024], ALU.add)
                else:
                    nc.gpsimd.tensor_tensor(zdst, sq[:, 0:512],
                                            sq[:, 512:1024], ALU.add)
                cnt["zadd"] += 1

            def av_alloc(qc, last):
                o_ps = [psA.tile([128, QCW], f32, tag=f"o{h}",
                                 name=f"o{h}_{qc}") for h in (0, 1)]
                if last:
                    rs = psA.tile([128, 1024], f32, tag="aux",
                                  name=f"auxrs_{qc}")
                else:
                    rs = work.tile([128, 1024], f16, tag="rs_acc",
                                   bufs=2, name=f"rsacc_{qc}")
                return o_ps, rs

            def av_tile(qc, bt, kt, o_ps, rs, last):
                for h in (0, 1):
                    vblk = v16_all[:, h * NK + kt * 128:
                                   h * NK + (kt + 1) * 128]
                    pcol = bt[:, kt * 1024 + h * 512:
                              kt * 1024 + h * 512 + 512]
                    nc.tensor.matmul(o_ps[h][:, :], vblk, pcol,
                                     start=(kt == 0), stop=(kt == KT - 1))
                pk = bt[:, kt * 1024:(kt + 1) * 1024]
                if last:
                    # rowsum via PE directly into aux psum row 0
                    # (split 2x512 so each matmul stays within one bank)
                    for cb in (0, 1):
                        nc.tensor.matmul(
                            rs[0:1, cb * 512:(cb + 1) * 512], ones16[:],
                            pk[:, cb * 512:(cb + 1) * 512],
                            start=(kt == 0), stop=(kt == KT - 1))
                else:
                    # rowsum accumulation on Pool (both heads at once)
                    if kt == 0:
                        nc.gpsimd.tensor_scalar_mul(rs[:], pk, 1.0)
                    else:
                        nc.gpsimd.tensor_tensor(rs[:], rs[:], pk, ALU.add)

            def tail_pre(qc, o_ps, rs, last):
                qs = slice(qc * QCW, qc * QCW + QCW)
                if last:
                    aux = rs
                else:
                    aux = psA.tile([128, 1024], f32, tag="aux",
                                   name=f"aux_{qc}")
                    for cb in (0, 1):
                        nc.tensor.matmul(aux[0:1, cb * 512:(cb + 1) * 512],
                                         ones16[:],
                                         rs[:, cb * 512:(cb + 1) * 512],
                                         start=True, stop=True)
                rcp = work.tile([1, 1024], f32r, tag="rcp",
                                name=f"rcp_{qc}")
                nc.vector.reciprocal(rcp[:], aux[0:1, :])
                bc = psA.tile([128, 1024], f32, tag="aux",
                              name=f"bc_{qc}")
                for cb in (0, 1):
                    nc.tensor.matmul(bc[:, cb * 512:(cb + 1) * 512],
                                     ones_row[:],
                                     rcp[:, cb * 512:(cb + 1) * 512],
                                     start=True, stop=True)
                bc_sb = work.tile([128, 1024], f32r, tag="bc_sb",
                                  name=f"bcsb_{qc}")
                nc.scalar.copy(bc_sb[:], bc[:])
                # normalize AV output into oT (psum -> f16 sbuf)
                for h in (0, 1):
                    for ri, dest in ((0, oT_re), (1, oT_im)):
                        rows = slice(64 * ri, 64 * ri + 64)
                        nc.vector.scalar_tensor_tensor(
                            dest[64 * h:64 * h + 64, qs],
                            o_ps[h][rows, :], 1.0,
                            bc_sb[rows, h * 512:h * 512 + 512],
                            ALU.mult, ALU.mult)

            def tail_post(qc):
                qs = slice(qc * QCW, qc * QCW + QCW)
                for Rc in range(4):
                    def wob(w):
                        return wop[:, w * 512 + Rc * 128:
                                   w * 512 + Rc * 128 + 128]

                    for ri, (wa, wb_, dst_e) in enumerate(
                            ((0, 2, ore_e), (1, 0, oim_e))):
                        wo = psA.tile([128, 512], f32, tag="sb", bufs=2,
                                      name=f"wo_{Rc}_{ri}_{qc}")
                        nc.tensor.matmul(wo[:], wob(wa), oT_re[:, qs],
                                         start=True, stop=False)
                        nc.tensor.matmul(wo[:], wob(wb_), oT_im[:, qs],
                                         start=False, stop=True)
                        st = work.tile([128, 512], f32, tag="st", bufs=4,
                                       name=f"st_{Rc}_{ri}_{qc}")
                        if cnt["stc"] % ST_MOD < ST_ACT_NUM:
                            nc.scalar.copy(st[:], wo[:])
                        else:
                            nc.vector.tensor_copy(st[:], wo[:])
                        cnt["stc"] += 1
                        nc.sync.dma_start(
                            dst_e[Rc * 128:(Rc + 1) * 128, qs], st[:])

            # ---- main schedule ----
            # Per q-chunk period: [A: AV+rowsum of qc-1 interleaved with
            # scores(qc, half0); sqrt(h0)] [B: tail of qc-1; scores(qc,
            # half1); sqrt(h1); exp(h0); exp(h1)]. ACT sees Square-only
            # between the two sqrts and the two exps -> 2 table loads/qc.
            for ncc in range(4):
                proj(ncc)
            pend = None
            for qc in range(QC):
                bt = work.tile([128, KT * 1024], f16, tag="batch",
                               bufs=2, name=f"bt_{qc}")
                if pend is not None:
                    pqc, pbt = pend
                    o_ps, rs = av_alloc(pqc, last=False)
                # A: scores(qc, h0-half k-tiles) + AV(pqc, first-half k-
                # tiles, gated on pqc's h0 exp which is already done)
                for k8 in range(HKT):
                    scores_tile(qc, k8, 0, bt)
                    scores_tile(qc, k8, 1, bt)
                    if pend is not None:
                        av_tile(pqc, pbt, k8, o_ps, rs, last=False)
                nc.scalar.activation(bt[:, 0:HKT * 1024],
                                     bt[:, 0:HKT * 1024], AF.Sqrt,
                                     scale=1.0 / 64.0)
                if qc == 0:
                    for ncc in range(4, 8):
                        proj(ncc)
                # B: scores(qc, h1-half) + AV(pqc, second-half k-tiles)
                for k8 in range(HKT):
                    scores_tile(qc, HKT + k8, 0, bt)
                    scores_tile(qc, HKT + k8, 1, bt)
                    if pend is not None:
                        av_tile(pqc, pbt, HKT + k8, o_ps, rs, last=False)
                nc.scalar.activation(bt[:, HKT * 1024:],
                                     bt[:, HKT * 1024:], AF.Sqrt,
                                     scale=1.0 / 64.0)
                if pend is not None:
                    tail_pre(pqc, o_ps, rs, last=False)
                nc.scalar.activation(bt[:, 0:HKT * 1024],
                                     bt[:, 0:HKT * 1024], AF.Exp,
                                     bias=eb_exp[:])
                nc.scalar.activation(bt[:, HKT * 1024:],
                                     bt[:, HKT * 1024:], AF.Exp,
                                     bias=eb_exp[:])
                if pend is not None:
                    tail_post(pqc)
                pend = (qc, bt)
            # final q-chunk: AV with PE rowsum (short tail), then tail
            pqc, pbt = pend
            o_ps, rs = av_alloc(pqc, last=True)
            for kt in range(KT):
                av_tile(pqc, pbt, kt, o_ps, rs, last=True)
            tail_pre(pqc, o_ps, rs, last=True)
            tail_post(pqc)

    nc.finalize()
    return nc


def _get_nc():
    if "nc" not in _CACHE:
        _CACHE["nc"] = _build_nc()
    return _CACHE["nc"]


def _core_inputs(c, inputs):
    b = c // 4
    h0 = 2 * (c % 4)
    hs = slice(h0 * 64, h0 * 64 + 128)

    xpack = np.empty((NCC, 24, 128, NCW), np.float16)
    for t, name in enumerate(
            ("Q_real", "Q_imag", "K_real", "K_imag", "V_real", "V_imag")):
        xT = np.ascontiguousarray(inputs[name][b].T)          # (512, 2048)
        xpack[:, t * 4:(t + 1) * 4] = (
            xT.reshape(4, 128, NCC, NCW).transpose(2, 0, 1, 3))

    wlist = []
    for kind in ("q", "A", "v"):
        base_r = inputs[{"q": "wq_r", "A": "wk_r", "v": "wv_r"}[kind]]
        base_i = inputs[{"q": "wq_i", "A": "wk_i", "v": "wv_i"}[kind]]
        for hh in (0, 1):
            rows = slice((h0 + hh) * 64, (h0 + hh) * 64 + 64)
            wr, wi_ = base_r[rows], base_i[rows]
            # rows of the projected tensor: [p_r; p_i]
            w1 = np.vstack([wr, wi_])        # x_re weights
            w2 = np.vstack([-wi_, wr])       # x_im weights
            wlist += [w1, w2]
    arr = np.empty((48, 128, 128), np.float16)
    for wi, mat in enumerate(wlist):
        arr[wi * 4:(wi + 1) * 4] = np.ascontiguousarray(mat.T).reshape(
            4, 128, 128)
    wpack = np.ascontiguousarray(arr.transpose(1, 0, 2)).reshape(
        128, 48 * 128)

    wo_r_T = np.ascontiguousarray(inputs["wo_r"][:, hs].T)    # (128, 512)
    wo_i_T = np.ascontiguousarray(inputs["wo_i"][:, hs].T)
    wopack = np.concatenate([wo_r_T, wo_i_T, -wo_i_T], axis=1)
    wopack = np.ascontiguousarray(wopack).astype(np.float16)

    return {
        "xpack": xpack,
        "wpack": wpack,
        "wopack": wopack,
        "onesr": np.ones((1, 128), np.float32),
        "ident": np.eye(128, dtype=np.float16),
    }


def kernel(**inputs):
    from concourse.bass_utils import run_bass_kernel_spmd

    nc = _get_nc()
    in_maps = [_core_inputs(c, inputs) for c in range(NCORES)]
    res = run_bass_kernel_spmd(nc, in_maps, list(range(NCORES)))
    out = np.empty((B, NQ, R, 2), np.float32)
    for b in range(B):
        re = np.zeros((512, NQ), np.float64)
        im = np.zeros((512, NQ), np.float64)
        for c in range(b * 4, b * 4 + 4):
            re += res.results[c]["out_re"]
            im += res.results[c]["out_im"]
        out[b, :, :, 0] = re.T
        out[b, :, :, 1] = im.T
    return out


# revision 30
# speedup vs baseline: 1.1708x; 1.0232x over previous
"""Complex-valued multi-head attention on 8 Trainium2 NeuronCores.

Sharding: batch(2) x head-pairs(4) -> 8 cores; each core runs one batch
element and 2 heads end-to-end (QKV proj -> complex scores -> |s| softmax
-> AV -> partial W_O), host sums the W_O partials over the 4 cores of each
batch element (tensor-parallel reduce) and transposes to the output layout.

v2 restructure vs baseline:
- score tiles are [128 kpos, 512 re | 512 im] per (ktile, head); one fused
  square pass (DVE tensor_tensor mult or ACT Square, mixed by ratio) reads
  the f32 PSUM pair and emits re^2|im^2 f16, replacing the old
  copy+mul+square chain.
- z = re^2+im^2 adds run on the otherwise-idle Pool engine (mixed with
  some DVE), rowsum accumulation also on Pool with a single PE ones-fold
  per q-chunk, freeing ~27us of PE rowsum matmuls.
- s_im uses q2 = [q_i; -q_r] derived by two cheap copies per n-chunk
  instead of a second projected K tensor (C_sb), dropping 128 projection
  matmuls.
- ACT table set switches 2x per q-chunk (9 loads total vs 25).
- projections for n-chunks 4..7 are emitted between the first q-chunk's
  halves so PE never idles during the attention warm-up.
"""
import sys

sys.path.insert(0, "/opt/trn_rl_repo")

import numpy as np

B, NQ, NK, R = 2, 2048, 2048, 512
H, DK, DV = 8, 64, 64
NCORES = 8
NCC = 8          # n-chunks for projection streaming (2048/256)
NCW = 256        # projection n-chunk width
QC = 4           # q-chunks in attention (2048/512)
QCW = 512
KT = 16          # k-tiles (2048/128)
HKT = 8          # k-tiles per half-batch

# engine-mix tuning. HW allows only ONE non-scalar PSUM operand per
# vector op, so the score extraction is either a single fused ACT Square
# (unary, one psum read) or a DVE copy + sbuf squares. ACT-fused tiles
# sit at the END of each half-stream where ACT is idle (at the start it
# is still finishing the previous half's sqrt/exp batches).
ACT_FUSED_FROM = 11   # tile_in_half >= this -> fused ACT square
ZADD_DVE_NUM = 0      # out of ZADD_MOD adds go to DVE
ZADD_MOD = 8
ST_ACT_NUM = 0        # out of ST_MOD W_O output copies go to ACT
ST_MOD = 2
PRJ_ACT = False       # projection copies on ACT
VT_ACT = True         # v-transpose copies on ACT
Q2_ACT = False        # q2 derivation on ACT

_CACHE = {}


def _build_nc():
    import concourse.bass as bass
    import concourse.tile as tile
    from concourse.tile import add_dep_helper
    from concourse import bacc, mybir

    f32 = mybir.dt.float32
    f32r = mybir.dt.float32r
    f16 = mybir.dt.float16
    ALU = mybir.AluOpType
    AF = mybir.ActivationFunctionType

    nc = bacc.Bacc("TRN2", target_bir_lowering=False, debug=False,
                   num_devices=NCORES)

    xpack_e = nc.dram_tensor("xpack", [NCC, 24, 128, NCW], f16,
                             kind="ExternalInput")
    wpack_e = nc.dram_tensor("wpack", [128, 48 * 128], f16,
                             kind="ExternalInput")
    wopack_e = nc.dram_tensor("wopack", [128, 3 * 512], f16,
                              kind="ExternalInput")
    onesr_e = nc.dram_tensor("onesr", [1, 128], f32r, kind="ExternalInput")
    ident_e = nc.dram_tensor("ident", [128, 128], f16, kind="ExternalInput")
    ore_e = nc.dram_tensor("out_re", [512, NQ], f32, kind="ExternalOutput")
    oim_e = nc.dram_tensor("out_im", [512, NQ], f32, kind="ExternalOutput")

    with tile.TileContext(nc) as tc:
      with nc.allow_low_precision(reason="fp16 softmax path"):
        with tc.tile_pool(name="pers", bufs=1) as pers, \
             tc.tile_pool(name="work", bufs=2) as work, \
             tc.tile_pool(name="psA", bufs=1, space="PSUM") as psA:

            # act-table loads are auto-inserted by Bacc.insert_act_table_loads
            # on the scheduled order; manual loads just get hoisted uselessly.

            # ---- constants ----
            wp = pers.tile([128, 48 * 128], f16, tag="wp")
            nc.sync.dma_start(wp[:], wpack_e[:])
            wop = pers.tile([128, 3 * 512], f16, tag="wop")
            nc.sync.dma_start(wop[:], wopack_e[:])
            ones_row = pers.tile([1, 128], f32r, tag="ones_row")
            nc.sync.dma_start(ones_row[:], onesr_e[:])
            ident16 = pers.tile([128, 128], f16, tag="ident16")
            nc.sync.dma_start(ident16[:], ident_e[:])
            ones16 = pers.tile([128, 1], f16, tag="ones16")
            nc.vector.memset(ones16[:], 1.0)
            eb_exp = pers.tile([128, 1], f32, tag="eb_exp")
            nc.vector.memset(eb_exp[:], -1.5)          # exp(mag - 1.5)

            # ---- projection destinations (h-major: cols h*2048 + n) ----
            q_all = pers.tile([128, 2 * NQ], f16, tag="q_all")
            q2_all = pers.tile([128, 2 * NQ], f16, tag="q2_all")
            A_all = pers.tile([128, 2 * NK], f16, tag="A_all")
            vT_all = pers.tile([128, 2 * NK], f16, tag="vT_all")
            v16_all = pers.tile([128, 2 * NK], f16, tag="v16_all")
            oT_re = pers.tile([128, NQ], f16, tag="oT_re")
            oT_im = pers.tile([128, NQ], f16, tag="oT_im")

            # spec s uses weight blocks 2s (x_re) and 2s+1 (x_im);
            # x-block index: t=0..1 q_re/q_im, 2..3 k, 4..5 v
            grp_dest = [q_all, A_all, vT_all]

            cnt = {"ext": 0, "zadd": 0, "prcp": 0, "stc": 0}

            def proj(ncc):
                xt = work.tile([128, 24 * NCW], f16, tag="xt",
                               name=f"xt_{ncc}")
                nc.sync.dma_start(
                    xt[:].rearrange("p (b f) -> p b f", f=NCW),
                    xpack_e[ncc].rearrange("b p f -> p b f"))

                def xblk(t, rc):
                    return xt[:, (t * 4 + rc) * NCW:(t * 4 + rc + 1) * NCW]

                def wblk(w, rc):
                    return wp[:, (w * 4 + rc) * 128:(w * 4 + rc + 1) * 128]

                cs0 = ncc * NCW
                for grp in range(3):
                    pj = psA.tile([128, 512], f32, tag="sb", bufs=2,
                                  name=f"pj_{ncc}_{grp}")
                    for sub in range(2):
                        s = grp * 2 + sub
                        tx = grp * 2
                        dst = pj[:, sub * 256:(sub + 1) * 256]
                        for rc in range(4):
                            nc.tensor.matmul(dst, wblk(2 * s, rc),
                                             xblk(tx, rc),
                                             start=(rc == 0), stop=False)
                        for rc in range(4):
                            nc.tensor.matmul(dst, wblk(2 * s + 1, rc),
                                             xblk(tx + 1, rc),
                                             start=False, stop=(rc == 3))
                    dest = grp_dest[grp]
                    dap = dest[:].rearrange("p (h n) -> p h n", h=2)[
                        :, :, cs0:cs0 + NCW]
                    pap = pj[:].rearrange("p (h n) -> p h n", h=2)
                    if PRJ_ACT:
                        nc.scalar.copy(dap, pap)
                    else:
                        nc.vector.tensor_copy(dap, pap)
                    cnt["prcp"] += 1
                # q2 = [q_i; -q_r] for this chunk (both heads)
                q2ap_t = q2_all[0:64].rearrange("p (h n) -> p h n", h=2)[
                    :, :, cs0:cs0 + NCW]
                qap_b = q_all[64:128].rearrange("p (h n) -> p h n", h=2)[
                    :, :, cs0:cs0 + NCW]
                if Q2_ACT:
                    nc.scalar.copy(q2ap_t, qap_b)
                else:
                    nc.vector.tensor_scalar_mul(q2ap_t, qap_b, 1.0)
                q2ap_b = q2_all[64:128].rearrange("p (h n) -> p h n", h=2)[
                    :, :, cs0:cs0 + NCW]
                qap_t = q_all[0:64].rearrange("p (h n) -> p h n", h=2)[
                    :, :, cs0:cs0 + NCW]
                if Q2_ACT:
                    nc.scalar.mul(q2ap_b, qap_t, -1.0)
                else:
                    nc.vector.tensor_scalar_mul(q2ap_b, qap_t, -1.0)
                # transpose this chunk's V columns
                for h in (0, 1):
                    for nt in (2 * ncc, 2 * ncc + 1):
                        blk = slice(h * NK + nt * 128,
                                    h * NK + (nt + 1) * 128)
                        vt_ps = psA.tile([128, 128], f16, tag=f"o{h}",
                                         name=f"vtp_{h}_{nt}")
                        nc.tensor.transpose(vt_ps[:], vT_all[:, blk],
                                            ident16[:])
                        if VT_ACT:
                            nc.scalar.copy(v16_all[:, blk], vt_ps[:])
                        else:
                            nc.vector.tensor_copy(v16_all[:, blk], vt_ps[:])

            # ---- attention helpers ----
            def scores_tile(qc, kt, h, bt):
                qs0 = qc * QCW
                ks = slice(h * NK + kt * 128, h * NK + (kt + 1) * 128)
                qsl = slice(h * NQ + qs0, h * NQ + qs0 + QCW)
                sb = psA.tile([128, 1024], f32, tag="sb", bufs=2,
                              name=f"sb_{qc}_{kt}_{h}")
                nc.tensor.matmul(sb[:, 0:512], A_all[:, ks],
                                 q_all[:, qsl], start=True, stop=True)
                nc.tensor.matmul(sb[:, 512:1024], A_all[:, ks],
                                 q2_all[:, qsl], start=True, stop=True)
                # square extraction: sq = sb*sb (re^2 | im^2), f16
                sq = work.tile([128, 1024], f16, tag="sq", bufs=4,
                               name=f"sq_{qc}_{kt}_{h}")
                tile_in_half = (kt % HKT) * 2 + h
                if tile_in_half >= ACT_FUSED_FROM:
                    # unary ACT square reads psum once -- legal and fused
                    nc.scalar.square(sq[:], sb[:])
                else:
                    # DVE path: one psum->sbuf copy, then sbuf squares
                    t16 = work.tile([128, 1024], f16, tag="t16", bufs=4,
                                    name=f"t16_{qc}_{kt}_{h}")
                    nc.vector.tensor_copy(t16[:], sb[:])
                    nc.vector.tensor_tensor(sq[:, 0:512], t16[:, 0:512],
                                            t16[:, 0:512], ALU.mult)
                    nc.gpsimd.tensor_tensor(sq[:, 512:1024],
                                            t16[:, 512:1024],
                                            t16[:, 512:1024], ALU.mult)
                cnt["ext"] += 1
                # z = re^2 + im^2 -> bt column slot
                zdst = bt[:, kt * 1024 + h * 512:kt * 1024 + h * 512 + 512]
                if ZADD_DVE_NUM and cnt["zadd"] % ZADD_MOD < ZADD_DVE_NUM:
                    nc.vector.tensor_tensor(zdst, sq[:, 0:512],
                                            sq[:, 512:1024], ALU.add)
                else:
                    nc.gpsimd.tensor_tensor(zdst, sq[:, 0:512],
                                            sq[:, 512:1024], ALU.add)
                cnt["zadd"] += 1

            def av_alloc(qc, last):
                o_ps = [psA.tile([128, QCW], f32, tag=f"o{h}",
                                 name=f"o{h}_{qc}") for h in (0, 1)]
                if last:
                    rs = psA.tile([128, 1024], f32, tag="aux",
                                  name=f"auxrs_{qc}")
                else:
                    rs = work.tile([128, 1024], f16, tag="rs_acc",
                                   bufs=2, name=f"rsacc_{qc}")
                return o_ps, rs

            def av_tile(qc, bt, kt, o_ps, rs, last):
                for h in (0, 1):
                    vblk = v16_all[:, h * NK + kt * 128:
                                   h * NK + (kt + 1) * 128]
                    pcol = bt[:, kt * 1024 + h * 512:
                              kt * 1024 + h * 512 + 512]
                    nc.tensor.matmul(o_ps[h][:, :], vblk, pcol,
                                     start=(kt == 0), stop=(kt == KT - 1))
                pk = bt[:, kt * 1024:(kt + 1) * 1024]
                if last:
                    # rowsum via PE directly into aux psum row 0
                    # (split 2x512 so each matmul stays within one bank)
                    for cb in (0, 1):
                        nc.tensor.matmul(
                            rs[0:1, cb * 512:(cb + 1) * 512], ones16[:],
                            pk[:, cb * 512:(cb + 1) * 512],
                            start=(kt == 0), stop=(kt == KT - 1))
                else:
                    # rowsum accumulation on Pool (both heads at once)
                    if kt == 0:
                        nc.gpsimd.tensor_scalar_mul(rs[:], pk, 1.0)
                    else:
                        nc.gpsimd.tensor_tensor(rs[:], rs[:], pk, ALU.add)

            def tail_pre(qc, o_ps, rs, last):
                qs = slice(qc * QCW, qc * QCW + QCW)
                if last:
                    aux = rs
                else:
                    aux = psA.tile([128, 1024], f32, tag="aux",
                                   name=f"aux_{qc}")
                    for cb in (0, 1):
                        nc.tensor.matmul(aux[0:1, cb * 512:(cb + 1) * 512],
                                         ones16[:],
                                         rs[:, cb * 512:(cb + 1) * 512],
                                         start=True, stop=True)
                rcp = work.tile([1, 1024], f32r, tag="rcp",
                                name=f"rcp_{qc}")
                nc.vector.reciprocal(rcp[:], aux[0:1, :])
                bc = psA.tile([128, 1024], f32, tag="aux",
                              name=f"bc_{qc}")
                for cb in (0, 1):
                    nc.tensor.matmul(bc[:, cb * 512:(cb + 1) * 512],
                                     ones_row[:],
                                     rcp[:, cb * 512:(cb + 1) * 512],
                                     start=True, stop=True)
                bc_sb = work.tile([128, 1024], f32r, tag="bc_sb",
                                  name=f"bcsb_{qc}")
                nc.scalar.copy(bc_sb[:], bc[:])
                # normalize AV output into oT (psum -> f16 sbuf)
                for h in (0, 1):
                    for ri, dest in ((0, oT_re), (1, oT_im)):
                        rows = slice(64 * ri, 64 * ri + 64)
                        nc.vector.scalar_tensor_tensor(
                            dest[64 * h:64 * h + 64, qs],
                            o_ps[h][rows, :], 1.0,
                            bc_sb[rows, h * 512:h * 512 + 512],
                            ALU.mult, ALU.mult)

            def tail_post(qc):
                qs = slice(qc * QCW, qc * QCW + QCW)
                for Rc in range(4):
                    def wob(w):
                        return wop[:, w * 512 + Rc * 128:
                                   w * 512 + Rc * 128 + 128]

                    for ri, (wa, wb_, dst_e) in enumerate(
                            ((0, 2, ore_e), (1, 0, oim_e))):
                        wo = psA.tile([128, 512], f32, tag="sb", bufs=2,
                                      name=f"wo_{Rc}_{ri}_{qc}")
                        nc.tensor.matmul(wo[:], wob(wa), oT_re[:, qs],
                                         start=True, stop=False)
                        nc.tensor.matmul(wo[:], wob(wb_), oT_im[:, qs],
                                         start=False, stop=True)
                        st = work.tile([128, 512], f32, tag="st", bufs=4,
                                       name=f"st_{Rc}_{ri}_{qc}")
                        if cnt["stc"] % ST_MOD < ST_ACT_NUM:
                            nc.scalar.copy(st[:], wo[:])
                        else:
                            nc.vector.tensor_copy(st[:], wo[:])
                        cnt["stc"] += 1
                        nc.sync.dma_start(
                            dst_e[Rc * 128:(Rc + 1) * 128, qs], st[:])

            # ---- main schedule ----
            # Per q-chunk period: [A: AV+rowsum of qc-1 interleaved with
            # scores(qc, half0); sqrt(h0)] [B: tail of qc-1; scores(qc,
            # half1); sqrt(h1); exp(h0); exp(h1)]. ACT sees Square-only
            # between the two sqrts and the two exps -> 2 table loads/qc.
            proj(0)
            proj(1)
            pend = None
            for qc in range(QC):
                bt = work.tile([128, KT * 1024], f16, tag="batch",
                               bufs=2, name=f"bt_{qc}")
                if pend is not None:
                    pqc, pbt = pend
                    o_ps, rs = av_alloc(pqc, last=False)
                # A: scores(qc, h0-half k-tiles) + AV(pqc, first-half k-
                # tiles, gated on pqc's h0 exp which is already done)
                proj_a = {2: 2, 4: 3, 5: 4, 6: 5}   # k8 -> ncc (qc0 only)
                for k8 in range(HKT):
                    if qc == 0 and k8 in proj_a:
                        proj(proj_a[k8])
                    scores_tile(qc, k8, 0, bt)
                    scores_tile(qc, k8, 1, bt)
                    if pend is not None:
                        av_tile(pqc, pbt, k8, o_ps, rs, last=False)
                nc.scalar.activation(bt[:, 0:HKT * 1024],
                                     bt[:, 0:HKT * 1024], AF.Sqrt,
                                     scale=1.0 / 64.0)
                if qc == QC - 1:
                    # last q-chunk: exp(h0) early so its AV can overlap
                    nc.scalar.activation(bt[:, 0:HKT * 1024],
                                         bt[:, 0:HKT * 1024], AF.Exp,
                                         bias=eb_exp[:])
                # B: scores(qc, h1-half) + AV(pqc, second-half k-tiles)
                proj_b = {0: 6, 2: 7}               # k8 -> ncc (qc0 only)
                for k8 in range(HKT):
                    if qc == 0 and k8 in proj_b:
                        proj(proj_b[k8])
                    scores_tile(qc, HKT + k8, 0, bt)
                    scores_tile(qc, HKT + k8, 1, bt)
                    if pend is not None:
                        av_tile(pqc, pbt, HKT + k8, o_ps, rs, last=False)
                nc.scalar.activation(bt[:, HKT * 1024:],
                                     bt[:, HKT * 1024:], AF.Sqrt,
                                     scale=1.0 / 64.0)
                if pend is not None:
                    tail_pre(pqc, o_ps, rs, last=False)
                if qc != QC - 1:
                    nc.scalar.activation(bt[:, 0:HKT * 1024],
                                         bt[:, 0:HKT * 1024], AF.Exp,
                                         bias=eb_exp[:])
                nc.scalar.activation(bt[:, HKT * 1024:],
                                     bt[:, HKT * 1024:], AF.Exp,
                                     bias=eb_exp[:])
                if pend is not None:
                    tail_post(pqc)
                pend = (qc, bt)
            # final q-chunk: AV with PE rowsum (short tail), then tail
            pqc, pbt = pend
            o_ps, rs = av_alloc(pqc, last=True)
            for kt in range(KT):
                av_tile(pqc, pbt, kt, o_ps, rs, last=True)
            tail_pre(pqc, o_ps, rs, last=True)
            tail_post(pqc)

    nc.finalize()
    return nc


def _get_nc():
    if "nc" not in _CACHE:
        _CACHE["nc"] = _build_nc()
    return _CACHE["nc"]


def _core_inputs(c, inputs):
    b = c // 4
    h0 = 2 * (c % 4)
    hs = slice(h0 * 64, h0 * 64 + 128)

    xpack = np.empty((NCC, 24, 128, NCW), np.float16)
    for t, name in enumerate(
            ("Q_real", "Q_imag", "K_real", "K_imag", "V_real", "V_imag")):
        xT = np.ascontiguousarray(inputs[name][b].T)          # (512, 2048)
        xpack[:, t * 4:(t + 1) * 4] = (
            xT.reshape(4, 128, NCC, NCW).transpose(2, 0, 1, 3))

    wlist = []
    for kind in ("q", "A", "v"):
        base_r = inputs[{"q": "wq_r", "A": "wk_r", "v": "wv_r"}[kind]]
        base_i = inputs[{"q": "wq_i", "A": "wk_i", "v": "wv_i"}[kind]]
        for hh in (0, 1):
            rows = slice((h0 + hh) * 64, (h0 + hh) * 64 + 64)
            wr, wi_ = base_r[rows], base_i[rows]
            # rows of the projected tensor: [p_r; p_i]
            w1 = np.vstack([wr, wi_])        # x_re weights
            w2 = np.vstack([-wi_, wr])       # x_im weights
            wlist += [w1, w2]
    arr = np.empty((48, 128, 128), np.float16)
    for wi, mat in enumerate(wlist):
        arr[wi * 4:(wi + 1) * 4] = np.ascontiguousarray(mat.T).reshape(
            4, 128, 128)
    wpack = np.ascontiguousarray(arr.transpose(1, 0, 2)).reshape(
        128, 48 * 128)

    wo_r_T = np.ascontiguousarray(inputs["wo_r"][:, hs].T)    # (128, 512)
    wo_i_T = np.ascontiguousarray(inputs["wo_i"][:, hs].T)
    wopack = np.concatenate([wo_r_T, wo_i_T, -wo_i_T], axis=1)
    wopack = np.ascontiguousarray(wopack).astype(np.float16)

    return {
        "xpack": xpack,
        "wpack": wpack,
        "wopack": wopack,
        "onesr": np.ones((1, 128), np.float32),
        "ident": np.eye(128, dtype=np.float16),
    }


def kernel(**inputs):
    from concourse.bass_utils import run_bass_kernel_spmd

    nc = _get_nc()
    in_maps = [_core_inputs(c, inputs) for c in range(NCORES)]
    res = run_bass_kernel_spmd(nc, in_maps, list(range(NCORES)))
    out = np.empty((B, NQ, R, 2), np.float32)
    for b in range(B):
        re = np.zeros((512, NQ), np.float64)
        im = np.zeros((512, NQ), np.float64)
        for c in range(b * 4, b * 4 + 4):
            re += res.results[c]["out_re"]
            im += res.results[c]["out_im"]
        out[b, :, :, 0] = re.T
        out[b, :, :, 1] = im.T
    return out


# revision 50
# speedup vs baseline: 1.2249x; 1.0462x over previous
"""Complex-valued multi-head attention on 8 Trainium2 NeuronCores.

Sharding: batch(2) x head-pairs(4) -> 8 cores; each core runs one batch
element and 2 heads end-to-end (QKV proj -> complex scores -> |s| softmax
-> AV -> partial W_O), host sums the W_O partials over the 4 cores of each
batch element (tensor-parallel reduce) and transposes to the output layout.

v2 restructure vs baseline:
- score tiles are [128 kpos, 512 re | 512 im] per (ktile, head); one fused
  square pass (DVE tensor_tensor mult or ACT Square, mixed by ratio) reads
  the f32 PSUM pair and emits re^2|im^2 f16, replacing the old
  copy+mul+square chain.
- z = re^2+im^2 adds run on the otherwise-idle Pool engine (mixed with
  some DVE), rowsum accumulation also on Pool with a single PE ones-fold
  per q-chunk, freeing ~27us of PE rowsum matmuls.
- s_im uses q2 = [q_i; -q_r] derived by two cheap copies per n-chunk
  instead of a second projected K tensor (C_sb), dropping 128 projection
  matmuls.
- ACT table set switches 2x per q-chunk (9 loads total vs 25).
- projections for n-chunks 4..7 are emitted between the first q-chunk's
  halves so PE never idles during the attention warm-up.
"""
import sys

sys.path.insert(0, "/opt/trn_rl_repo")

import numpy as np

B, NQ, NK, R = 2, 2048, 2048, 512
H, DK, DV = 8, 64, 64
NCORES = 8
NCC = 8          # n-chunks for projection streaming (2048/256)
NCW = 256        # projection n-chunk width
QC = 4           # q-chunks in attention (2048/512)
QCW = 512
KT = 16          # k-tiles (2048/128)
HKT = 8          # k-tiles per half-batch

# engine-mix tuning. HW allows only ONE non-scalar PSUM operand per
# vector op, so the score extraction is either a single fused ACT Square
# (unary, one psum read) or a DVE copy + sbuf squares. ACT-fused tiles
# sit at the END of each half-stream where ACT is idle (at the start it
# is still finishing the previous half's sqrt/exp batches).
ACT_FUSED_FROM = 12   # tile_in_half >= this -> fused ACT square
ZADD_DVE_NUM = 0      # out of ZADD_MOD adds go to DVE
ZADD_MOD = 8
ST_ACT_NUM = 0        # out of ST_MOD W_O output copies go to ACT
ST_MOD = 2
PRJ_ACT = False       # projection copies on ACT
VT_ACT = True         # v-transpose copies on ACT
Q2_ACT = False        # q2 derivation on ACT

_CACHE = {}


def _build_nc():
    import concourse.bass as bass
    import concourse.tile as tile
    from concourse.tile import add_dep_helper
    from concourse import bacc, mybir

    f32 = mybir.dt.float32
    f32r = mybir.dt.float32r
    f16 = mybir.dt.float16
    ALU = mybir.AluOpType
    AF = mybir.ActivationFunctionType

    nc = bacc.Bacc("TRN2", target_bir_lowering=False, debug=False,
                   num_devices=NCORES)

    xpack_e = nc.dram_tensor("xpack", [NCC, 24, 128, NCW], f16,
                             kind="ExternalInput")
    wpack_e = nc.dram_tensor("wpack", [128, 48 * 128], f16,
                             kind="ExternalInput")
    wopack_e = nc.dram_tensor("wopack", [128, 3 * 512], f16,
                              kind="ExternalInput")
    onesr_e = nc.dram_tensor("onesr", [1, 128], f32r, kind="ExternalInput")
    ident_e = nc.dram_tensor("ident", [128, 128], f16, kind="ExternalInput")
    ore_e = nc.dram_tensor("out_re", [512, NQ], f32, kind="ExternalOutput")
    oim_e = nc.dram_tensor("out_im", [512, NQ], f32, kind="ExternalOutput")

    with tile.TileContext(nc) as tc:
      with nc.allow_low_precision(reason="fp16 softmax path"):
        with tc.tile_pool(name="pers", bufs=1) as pers, \
             tc.tile_pool(name="work", bufs=2) as work, \
             tc.tile_pool(name="psA", bufs=1, space="PSUM") as psA:

            # act-table loads are auto-inserted by Bacc.insert_act_table_loads
            # on the scheduled order; manual loads just get hoisted uselessly.

            # ---- constants ----
            wp = pers.tile([128, 48 * 128], f16, tag="wp")
            nc.sync.dma_start(wp[:], wpack_e[:])
            wop = pers.tile([128, 3 * 512], f16, tag="wop")
            nc.sync.dma_start(wop[:], wopack_e[:])
            ones_row = pers.tile([1, 128], f32r, tag="ones_row")
            nc.sync.dma_start(ones_row[:], onesr_e[:])
            ident16 = pers.tile([128, 128], f16, tag="ident16")
            nc.sync.dma_start(ident16[:], ident_e[:])
            ones16 = pers.tile([128, 1], f16, tag="ones16")
            nc.vector.memset(ones16[:], 1.0)
            eb_exp = pers.tile([128, 1], f32, tag="eb_exp")
            nc.vector.memset(eb_exp[:], -1.5)          # exp(mag - 1.5)

            # ---- projection destinations (h-major: cols h*2048 + n) ----
            q_all = pers.tile([128, 2 * NQ], f16, tag="q_all")
            q2_all = pers.tile([128, 2 * NQ], f16, tag="q2_all")
            A_all = pers.tile([128, 2 * NK], f16, tag="A_all")
            vT_all = pers.tile([128, 2 * NK], f16, tag="vT_all")
            v16_all = pers.tile([128, 2 * NK], f16, tag="v16_all")
            oT_re = pers.tile([128, NQ], f16, tag="oT_re")
            oT_im = pers.tile([128, NQ], f16, tag="oT_im")

            # spec s uses weight blocks 2s (x_re) and 2s+1 (x_im);
            # x-block index: t=0..1 q_re/q_im, 2..3 k, 4..5 v
            grp_dest = [q_all, A_all, vT_all]

            cnt = {"ext": 0, "zadd": 0, "prcp": 0, "stc": 0}

            def proj(ncc):
                xt = work.tile([128, 24 * NCW], f16, tag="xt",
                               name=f"xt_{ncc}")
                nc.sync.dma_start(
                    xt[:].rearrange("p (b f) -> p b f", f=NCW),
                    xpack_e[ncc].rearrange("b p f -> p b f"))

                def xblk(t, rc):
                    return xt[:, (t * 4 + rc) * NCW:(t * 4 + rc + 1) * NCW]

                def wblk(w, rc):
                    return wp[:, (w * 4 + rc) * 128:(w * 4 + rc + 1) * 128]

                cs0 = ncc * NCW
                for grp in range(3):
                    pj = psA.tile([128, 512], f32, tag="sb", bufs=2,
                                  name=f"pj_{ncc}_{grp}")
                    for sub in range(2):
                        s = grp * 2 + sub
                        tx = grp * 2
                        dst = pj[:, sub * 256:(sub + 1) * 256]
                        for rc in range(4):
                            nc.tensor.matmul(dst, wblk(2 * s, rc),
                                             xblk(tx, rc),
                                             start=(rc == 0), stop=False)
                        for rc in range(4):
                            nc.tensor.matmul(dst, wblk(2 * s + 1, rc),
                                             xblk(tx + 1, rc),
                                             start=False, stop=(rc == 3))
                    dest = grp_dest[grp]
                    dap = dest[:].rearrange("p (h n) -> p h n", h=2)[
                        :, :, cs0:cs0 + NCW]
                    pap = pj[:].rearrange("p (h n) -> p h n", h=2)
                    if PRJ_ACT:
                        nc.scalar.copy(dap, pap)
                    else:
                        nc.vector.tensor_copy(dap, pap)
                    cnt["prcp"] += 1
                # q2 = [q_i; -q_r] for this chunk (both heads)
                q2ap_t = q2_all[0:64].rearrange("p (h n) -> p h n", h=2)[
                    :, :, cs0:cs0 + NCW]
                qap_b = q_all[64:128].rearrange("p (h n) -> p h n", h=2)[
                    :, :, cs0:cs0 + NCW]
                if Q2_ACT:
                    nc.scalar.copy(q2ap_t, qap_b)
                else:
                    nc.vector.tensor_scalar_mul(q2ap_t, qap_b, 1.0)
                q2ap_b = q2_all[64:128].rearrange("p (h n) -> p h n", h=2)[
                    :, :, cs0:cs0 + NCW]
                qap_t = q_all[0:64].rearrange("p (h n) -> p h n", h=2)[
                    :, :, cs0:cs0 + NCW]
                if Q2_ACT:
                    nc.scalar.mul(q2ap_b, qap_t, -1.0)
                else:
                    nc.vector.tensor_scalar_mul(q2ap_b, qap_t, -1.0)
                # transpose this chunk's V columns
                for h in (0, 1):
                    for nt in (2 * ncc, 2 * ncc + 1):
                        blk = slice(h * NK + nt * 128,
                                    h * NK + (nt + 1) * 128)
                        vt_ps = psA.tile([128, 128], f16, tag=f"o{h}",
                                         name=f"vtp_{h}_{nt}")
                        nc.tensor.transpose(vt_ps[:], vT_all[:, blk],
                                            ident16[:])
                        if VT_ACT:
                            nc.scalar.copy(v16_all[:, blk], vt_ps[:])
                        else:
                            nc.vector.tensor_copy(v16_all[:, blk], vt_ps[:])

            # ---- attention helpers ----
            def scores_tile(qc, kt, h, bt):
                qs0 = qc * QCW
                ks = slice(h * NK + kt * 128, h * NK + (kt + 1) * 128)
                qsl = slice(h * NQ + qs0, h * NQ + qs0 + QCW)
                sb = psA.tile([128, 1024], f32, tag="sb", bufs=2,
                              name=f"sb_{qc}_{kt}_{h}")
                nc.tensor.matmul(sb[:, 0:512], A_all[:, ks],
                                 q_all[:, qsl], start=True, stop=True)
                nc.tensor.matmul(sb[:, 512:1024], A_all[:, ks],
                                 q2_all[:, qsl], start=True, stop=True)
                # square extraction: sq = sb*sb (re^2 | im^2), f16
                sq = work.tile([128, 1024], f16, tag="sq", bufs=6,
                               name=f"sq_{qc}_{kt}_{h}")
                tile_in_half = (kt % HKT) * 2 + h
                if tile_in_half >= ACT_FUSED_FROM:
                    # unary ACT square reads psum once -- legal and fused
                    nc.scalar.square(sq[:], sb[:])
                else:
                    # DVE path: one psum->sbuf copy, then sbuf squares
                    t16 = work.tile([128, 1024], f16, tag="t16", bufs=6,
                                    name=f"t16_{qc}_{kt}_{h}")
                    nc.vector.tensor_copy(t16[:], sb[:])
                    nc.vector.tensor_tensor(sq[:, 0:512], t16[:, 0:512],
                                            t16[:, 0:512], ALU.mult)
                    nc.gpsimd.tensor_tensor(sq[:, 512:1024],
                                            t16[:, 512:1024],
                                            t16[:, 512:1024], ALU.mult)
                cnt["ext"] += 1
                # z = re^2 + im^2 -> bt column slot
                zdst = bt[:, kt * 1024 + h * 512:kt * 1024 + h * 512 + 512]
                if ZADD_DVE_NUM and cnt["zadd"] % ZADD_MOD < ZADD_DVE_NUM:
                    nc.vector.tensor_tensor(zdst, sq[:, 0:512],
                                            sq[:, 512:1024], ALU.add)
                else:
                    nc.gpsimd.tensor_tensor(zdst, sq[:, 0:512],
                                            sq[:, 512:1024], ALU.add)
                cnt["zadd"] += 1

            def av_alloc(qc, last):
                o_ps = [psA.tile([128, QCW], f32, tag=f"o{h}",
                                 name=f"o{h}_{qc}") for h in (0, 1)]
                if last:
                    rs = psA.tile([128, 1024], f32, tag="aux",
                                  name=f"auxrs_{qc}")
                else:
                    rs = work.tile([128, 1024], f16, tag="rs_acc",
                                   bufs=2, name=f"rsacc_{qc}")
                return o_ps, rs

            def av_tile(qc, bt, kt, o_ps, rs, last):
                for h in (0, 1):
                    vblk = v16_all[:, h * NK + kt * 128:
                                   h * NK + (kt + 1) * 128]
                    pcol = bt[:, kt * 1024 + h * 512:
                              kt * 1024 + h * 512 + 512]
                    nc.tensor.matmul(o_ps[h][:, :], vblk, pcol,
                                     start=(kt == 0), stop=(kt == KT - 1))
                pk = bt[:, kt * 1024:(kt + 1) * 1024]
                if last:
                    # rowsum via PE directly into aux psum row 0
                    # (split 2x512 so each matmul stays within one bank)
                    for cb in (0, 1):
                        nc.tensor.matmul(
                            rs[0:1, cb * 512:(cb + 1) * 512], ones16[:],
                            pk[:, cb * 512:(cb + 1) * 512],
                            start=(kt == 0), stop=(kt == KT - 1))
                else:
                    # rowsum accumulation on Pool (both heads at once)
                    if kt == 0:
                        nc.gpsimd.tensor_scalar_mul(rs[:], pk, 1.0)
                    else:
                        nc.gpsimd.tensor_tensor(rs[:], rs[:], pk, ALU.add)

            def tail_pre(qc, o_ps, rs, last):
                qs = slice(qc * QCW, qc * QCW + QCW)
                if last:
                    aux = rs
                else:
                    aux = psA.tile([128, 1024], f32, tag="aux",
                                   name=f"aux_{qc}")
                    for cb in (0, 1):
                        nc.tensor.matmul(aux[0:1, cb * 512:(cb + 1) * 512],
                                         ones16[:],
                                         rs[:, cb * 512:(cb + 1) * 512],
                                         start=True, stop=True)
                rcp = work.tile([1, 1024], f32r, tag="rcp",
                                name=f"rcp_{qc}")
                nc.vector.reciprocal(rcp[:], aux[0:1, :])
                bc = psA.tile([128, 1024], f32, tag="aux",
                              name=f"bc_{qc}")
                for cb in (0, 1):
                    nc.tensor.matmul(bc[:, cb * 512:(cb + 1) * 512],
                                     ones_row[:],
                                     rcp[:, cb * 512:(cb + 1) * 512],
                                     start=True, stop=True)
                bc_sb = work.tile([128, 1024], f32r, tag="bc_sb",
                                  name=f"bcsb_{qc}")
                nc.scalar.copy(bc_sb[:], bc[:])
                # normalize AV output into oT (psum -> f16 sbuf)
                for h in (0, 1):
                    for ri, dest in ((0, oT_re), (1, oT_im)):
                        rows = slice(64 * ri, 64 * ri + 64)
                        nc.vector.scalar_tensor_tensor(
                            dest[64 * h:64 * h + 64, qs],
                            o_ps[h][rows, :], 1.0,
                            bc_sb[rows, h * 512:h * 512 + 512],
                            ALU.mult, ALU.mult)

            def tail_post(qc):
                qs = slice(qc * QCW, qc * QCW + QCW)
                for Rc in range(4):
                    def wob(w):
                        return wop[:, w * 512 + Rc * 128:
                                   w * 512 + Rc * 128 + 128]

                    for ri, (wa, wb_, dst_e) in enumerate(
                            ((0, 2, ore_e), (1, 0, oim_e))):
                        wo = psA.tile([128, 512], f32, tag="sb", bufs=2,
                                      name=f"wo_{Rc}_{ri}_{qc}")
                        nc.tensor.matmul(wo[:], wob(wa), oT_re[:, qs],
                                         start=True, stop=False)
                        nc.tensor.matmul(wo[:], wob(wb_), oT_im[:, qs],
                                         start=False, stop=True)
                        st = work.tile([128, 512], f32, tag="st", bufs=6,
                                       name=f"st_{Rc}_{ri}_{qc}")
                        if qc == QC - 1:
                            act_st = cnt["stc"] % 2 == 0  # parallel tail
                        else:
                            act_st = cnt["stc"] % ST_MOD < ST_ACT_NUM
                        if act_st:
                            nc.scalar.copy(st[:], wo[:])
                        else:
                            nc.vector.tensor_copy(st[:], wo[:])
                        cnt["stc"] += 1
                        nc.sync.dma_start(
                            dst_e[Rc * 128:(Rc + 1) * 128, qs], st[:])

            # ---- main schedule ----
            # Per q-chunk period: [A: AV+rowsum of qc-1 interleaved with
            # scores(qc, half0); sqrt(h0)] [B: tail of qc-1; scores(qc,
            # half1); sqrt(h1); exp(h0); exp(h1)]. ACT sees Square-only
            # between the two sqrts and the two exps -> 2 table loads/qc.
            proj(0)
            proj(1)
            pend = None
            for qc in range(QC):
                bt = work.tile([128, KT * 1024], f16, tag="batch",
                               bufs=2, name=f"bt_{qc}")
                if pend is not None:
                    pqc, pbt = pend
                    o_ps, rs = av_alloc(pqc, last=False)
                # A: scores(qc, h0-half k-tiles) + AV(pqc, first-half k-
                # tiles, gated on pqc's h0 exp which is already done)
                proj_a = {2: 2, 4: 3, 5: 4, 6: 5}   # k8 -> ncc (qc0 only)
                for k8 in range(HKT):
                    if qc == 0 and k8 in proj_a:
                        proj(proj_a[k8])
                    scores_tile(qc, k8, 0, bt)
                    scores_tile(qc, k8, 1, bt)
                    if pend is not None:
                        av_tile(pqc, pbt, k8, o_ps, rs, last=False)
                nc.scalar.activation(bt[:, 0:HKT * 1024],
                                     bt[:, 0:HKT * 1024], AF.Sqrt,
                                     scale=1.0 / 64.0)
                if qc == QC - 1:
                    # last q-chunk: exp(h0) early so its AV can overlap
                    nc.scalar.activation(bt[:, 0:HKT * 1024],
                                         bt[:, 0:HKT * 1024], AF.Exp,
                                         bias=eb_exp[:])
                # B: scores(qc, h1-half) + AV(pqc, second-half k-tiles)
                proj_b = {0: 6, 2: 7}               # k8 -> ncc (qc0 only)
                for k8 in range(HKT):
                    if qc == 0 and k8 in proj_b:
                        proj(proj_b[k8])
                    scores_tile(qc, HKT + k8, 0, bt)
                    scores_tile(qc, HKT + k8, 1, bt)
                    if pend is not None:
                        av_tile(pqc, pbt, HKT + k8, o_ps, rs, last=False)
                nc.scalar.activation(bt[:, HKT * 1024:],
                                     bt[:, HKT * 1024:], AF.Sqrt,
                                     scale=1.0 / 64.0)
                if pend is not None:
                    tail_pre(pqc, o_ps, rs, last=False)
                if qc != QC - 1:
                    for hh in range(4):
                        cs = slice(hh * 4096, (hh + 1) * 4096)
                        nc.scalar.activation(bt[:, cs], bt[:, cs], AF.Exp,
                                             bias=eb_exp[:])
                else:
                    # split the last exp so the final AV can start on the
                    # first quarter while the rest is still computing
                    for qtr in range(4):
                        cs = slice(HKT * 1024 + qtr * 2048,
                                   HKT * 1024 + (qtr + 1) * 2048)
                        nc.scalar.activation(bt[:, cs], bt[:, cs], AF.Exp,
                                             bias=eb_exp[:])
                if pend is not None:
                    tail_post(pqc)
                pend = (qc, bt)
            # final q-chunk: AV with PE rowsum (short tail), then tail
            pqc, pbt = pend
            o_ps, rs = av_alloc(pqc, last=True)
            for kt in range(KT):
                av_tile(pqc, pbt, kt, o_ps, rs, last=True)
            tail_pre(pqc, o_ps, rs, last=True)
            tail_post(pqc)

    nc.finalize()
    return nc


def _get_nc():
    if "nc" not in _CACHE:
        _CACHE["nc"] = _build_nc()
    return _CACHE["nc"]


def _core_inputs(c, inputs):
    b = c // 4
    h0 = 2 * (c % 4)
    hs = slice(h0 * 64, h0 * 64 + 128)

    xpack = np.empty((NCC, 24, 128, NCW), np.float16)
    for t, name in enumerate(
            ("Q_real", "Q_imag", "K_real", "K_imag", "V_real", "V_imag")):
        xT = np.ascontiguousarray(inputs[name][b].T)          # (512, 2048)
        xpack[:, t * 4:(t + 1) * 4] = (
            xT.reshape(4, 128, NCC, NCW).transpose(2, 0, 1, 3))

    wlist = []
    for kind in ("q", "A", "v"):
        base_r = inputs[{"q": "wq_r", "A": "wk_r", "v": "wv_r"}[kind]]
        base_i = inputs[{"q": "wq_i", "A": "wk_i", "v": "wv_i"}[kind]]
        for hh in (0, 1):
            rows = slice((h0 + hh) * 64, (h0 + hh) * 64 + 64)
            wr, wi_ = base_r[rows], base_i[rows]
            # rows of the projected tensor: [p_r; p_i]
            w1 = np.vstack([wr, wi_])        # x_re weights
            w2 = np.vstack([-wi_, wr])       # x_im weights
            wlist += [w1, w2]
    arr = np.empty((48, 128, 128), np.float16)
    for wi, mat in enumerate(wlist):
        arr[wi * 4:(wi + 1) * 4] = np.ascontiguousarray(mat.T).reshape(
            4, 128, 128)
    wpack = np.ascontiguousarray(arr.transpose(1, 0, 2)).reshape(
        128, 48 * 128)

    wo_r_T = np.ascontiguousarray(inputs["wo_r"][:, hs].T)    # (128, 512)
    wo_i_T = np.ascontiguousarray(inputs["wo_i"][:, hs].T)
    wopack = np.concatenate([wo_r_T, wo_i_T, -wo_i_T], axis=1)
    wopack = np.ascontiguousarray(wopack).astype(np.float16)

    return {
        "xpack": xpack,
        "wpack": wpack,
        "wopack": wopack,
        "onesr": np.ones((1, 128), np.float32),
        "ident": np.eye(128, dtype=np.float16),
    }


def kernel(**inputs):
    from concourse.bass_utils import run_bass_kernel_spmd

    nc = _get_nc()
    in_maps = [_core_inputs(c, inputs) for c in range(NCORES)]
    res = run_bass_kernel_spmd(nc, in_maps, list(range(NCORES)))
    out = np.empty((B, NQ, R, 2), np.float32)
    for b in range(B):
        re = np.zeros((512, NQ), np.float64)
        im = np.zeros((512, NQ), np.float64)
        for c in range(b * 4, b * 4 + 4):
            re += res.results[c]["out_re"]
            im += res.results[c]["out_im"]
        out[b, :, :, 0] = re.T
        out[b, :, :, 1] = im.T
    return out


# revision 59
# speedup vs baseline: 1.2249x; 1.0000x over previous
"""Complex-valued multi-head attention on 8 Trainium2 NeuronCores.

Sharding: batch(2) x head-pairs(4) -> 8 cores; each core runs one batch
element and 2 heads end-to-end (QKV proj -> complex scores -> |s| softmax
-> AV -> partial W_O), host sums the W_O partials over the 4 cores of each
batch element (tensor-parallel reduce) and transposes to the output layout.

Restructure vs the original baseline (340881ns -> 278291ns cost-model):
- score tiles are [128 kpos, 512 re | 512 im] per (ktile, head). HW allows
  only one PSUM operand per vector op, so extraction is either a fused
  unary ACT Square (tiles at the end of each half-stream, where ACT is
  otherwise idle) or a DVE psum->f16 copy + sbuf square (DVE) + sbuf
  square (Pool) for the rest.
- z = re^2+im^2 adds and the softmax rowsum accumulation run on the
  otherwise-idle Pool engine (one PE ones-fold per q-chunk); the last
  q-chunk uses PE ones-matmul rowsums to shorten the tail.
- s_im = A^T q2 with q2 = [q_i; -q_r] derived by two cheap copies per
  n-chunk instead of a second projected K tensor, dropping 128
  projection matmuls.
- sqrt/exp batch per half q-chunk ([128, 8192]); exps split 4x4096 so AV
  can start on early k-tiles; act-table loads are left to the framework
  pass (manual loads just get hoisted).
- emission interleaves the previous q-chunk's AV/rowsum into the current
  score stream (first-half AV with first-half scores etc.) and the
  normalization/W_O tail between the two halves, so no engine ever
  head-of-line blocks on the big ACT batches; projections are interleaved
  with the first q-chunk's scores at matching n-chunk granularity.
"""
import sys

sys.path.insert(0, "/opt/trn_rl_repo")

import numpy as np

B, NQ, NK, R = 2, 2048, 2048, 512
H, DK, DV = 8, 64, 64
NCORES = 8
NCC = 8          # n-chunks for projection streaming (2048/256)
NCW = 256        # projection n-chunk width
QC = 4           # q-chunks in attention (2048/512)
QCW = 512
KT = 16          # k-tiles (2048/128)
HKT = 8          # k-tiles per half-batch

# engine-mix tuning. HW allows only ONE non-scalar PSUM operand per
# vector op, so the score extraction is either a single fused ACT Square
# (unary, one psum read) or a DVE copy + sbuf squares. ACT-fused tiles
# sit at the END of each half-stream where ACT is idle (at the start it
# is still finishing the previous half's sqrt/exp batches).
MIXED_FROM = 12       # tile_in_half in [MIXED_FROM, ACT_FUSED_FROM) -> mixed
ACT_FUSED_FROM = 12   # tile_in_half >= this -> fused ACT square
ZADD_DVE_NUM = 0      # out of ZADD_MOD adds go to DVE
ZADD_MOD = 8
ST_ACT_NUM = 0        # out of ST_MOD W_O output copies go to ACT
ST_MOD = 2
PRJ_ACT = False       # projection copies on ACT
VT_ACT = True         # v-transpose copies on ACT
Q2_ACT = False        # q2 derivation on ACT

_CACHE = {}


def _build_nc():
    import concourse.bass as bass
    import concourse.tile as tile
    from concourse.tile import add_dep_helper
    from concourse import bacc, mybir

    f32 = mybir.dt.float32
    f32r = mybir.dt.float32r
    f16 = mybir.dt.float16
    ALU = mybir.AluOpType
    AF = mybir.ActivationFunctionType

    nc = bacc.Bacc("TRN2", target_bir_lowering=False, debug=False,
                   num_devices=NCORES)

    xpack_e = nc.dram_tensor("xpack", [NCC, 24, 128, NCW], f16,
                             kind="ExternalInput")
    wpack_e = nc.dram_tensor("wpack", [128, 48 * 128], f16,
                             kind="ExternalInput")
    wopack_e = nc.dram_tensor("wopack", [128, 3 * 512], f16,
                              kind="ExternalInput")
    onesr_e = nc.dram_tensor("onesr", [1, 128], f32r, kind="ExternalInput")
    ident_e = nc.dram_tensor("ident", [128, 128], f16, kind="ExternalInput")
    ore_e = nc.dram_tensor("out_re", [512, NQ], f32, kind="ExternalOutput")
    oim_e = nc.dram_tensor("out_im", [512, NQ], f32, kind="ExternalOutput")

    with tile.TileContext(nc) as tc:
      with nc.allow_low_precision(reason="fp16 softmax path"):
        with tc.tile_pool(name="pers", bufs=1) as pers, \
             tc.tile_pool(name="work", bufs=2) as work, \
             tc.tile_pool(name="psA", bufs=1, space="PSUM") as psA:

            # act-table loads are auto-inserted by Bacc.insert_act_table_loads
            # on the scheduled order; manual loads just get hoisted uselessly.

            # ---- constants ----
            wp = pers.tile([128, 48 * 128], f16, tag="wp")
            for wseg in range(3):
                ws = slice(wseg * 2048, (wseg + 1) * 2048)
                nc.sync.dma_start(wp[:, ws], wpack_e[:, ws])
            wop = pers.tile([128, 3 * 512], f16, tag="wop")
            nc.sync.dma_start(wop[:], wopack_e[:])
            ones_row = pers.tile([1, 128], f32r, tag="ones_row")
            nc.sync.dma_start(ones_row[:], onesr_e[:])
            ident16 = pers.tile([128, 128], f16, tag="ident16")
            nc.sync.dma_start(ident16[:], ident_e[:])
            ones16 = pers.tile([128, 1], f16, tag="ones16")
            nc.vector.memset(ones16[:], 1.0)
            eb_exp = pers.tile([128, 1], f32, tag="eb_exp")
            nc.vector.memset(eb_exp[:], -1.5)          # exp(mag - 1.5)

            # ---- projection destinations (h-major: cols h*2048 + n) ----
            q_all = pers.tile([128, 2 * NQ], f16, tag="q_all")
            q2_all = pers.tile([128, 2 * NQ], f16, tag="q2_all")
            A_all = pers.tile([128, 2 * NK], f16, tag="A_all")
            vT_all = pers.tile([128, 2 * NK], f16, tag="vT_all")
            v16_all = pers.tile([128, 2 * NK], f16, tag="v16_all")
            oT_re = pers.tile([128, NQ], f16, tag="oT_re")
            oT_im = pers.tile([128, NQ], f16, tag="oT_im")

            # spec s uses weight blocks 2s (x_re) and 2s+1 (x_im);
            # x-block index: t=0..1 q_re/q_im, 2..3 k, 4..5 v
            grp_dest = [q_all, A_all, vT_all]

            cnt = {"ext": 0, "zadd": 0, "prcp": 0, "stc": 0}

            def proj(ncc):
                xt = work.tile([128, 24 * NCW], f16, tag="xt",
                               name=f"xt_{ncc}")
                nc.sync.dma_start(
                    xt[:].rearrange("p (b f) -> p b f", f=NCW),
                    xpack_e[ncc].rearrange("b p f -> p b f"))

                def xblk(t, rc):
                    return xt[:, (t * 4 + rc) * NCW:(t * 4 + rc + 1) * NCW]

                def wblk(w, rc):
                    return wp[:, (w * 4 + rc) * 128:(w * 4 + rc + 1) * 128]

                cs0 = ncc * NCW
                for grp in range(3):
                    pj = psA.tile([128, 512], f32, tag="sb", bufs=2,
                                  name=f"pj_{ncc}_{grp}")
                    for sub in range(2):
                        s = grp * 2 + sub
                        tx = grp * 2
                        dst = pj[:, sub * 256:(sub + 1) * 256]
                        for rc in range(4):
                            nc.tensor.matmul(dst, wblk(2 * s, rc),
                                             xblk(tx, rc),
                                             start=(rc == 0), stop=False)
                        for rc in range(4):
                            nc.tensor.matmul(dst, wblk(2 * s + 1, rc),
                                             xblk(tx + 1, rc),
                                             start=False, stop=(rc == 3))
                    dest = grp_dest[grp]
                    dap = dest[:].rearrange("p (h n) -> p h n", h=2)[
                        :, :, cs0:cs0 + NCW]
                    pap = pj[:].rearrange("p (h n) -> p h n", h=2)
                    if PRJ_ACT:
                        nc.scalar.copy(dap, pap)
                    else:
                        nc.vector.tensor_copy(dap, pap)
                    cnt["prcp"] += 1
                # q2 = [q_i; -q_r] for this chunk (both heads)
                q2ap_t = q2_all[0:64].rearrange("p (h n) -> p h n", h=2)[
                    :, :, cs0:cs0 + NCW]
                qap_b = q_all[64:128].rearrange("p (h n) -> p h n", h=2)[
                    :, :, cs0:cs0 + NCW]
                if Q2_ACT:
                    nc.scalar.copy(q2ap_t, qap_b)
                else:
                    nc.vector.tensor_scalar_mul(q2ap_t, qap_b, 1.0)
                q2ap_b = q2_all[64:128].rearrange("p (h n) -> p h n", h=2)[
                    :, :, cs0:cs0 + NCW]
                qap_t = q_all[0:64].rearrange("p (h n) -> p h n", h=2)[
                    :, :, cs0:cs0 + NCW]
                if Q2_ACT:
                    nc.scalar.mul(q2ap_b, qap_t, -1.0)
                else:
                    nc.vector.tensor_scalar_mul(q2ap_b, qap_t, -1.0)
                # transpose this chunk's V columns
                for h in (0, 1):
                    for nt in (2 * ncc, 2 * ncc + 1):
                        blk = slice(h * NK + nt * 128,
                                    h * NK + (nt + 1) * 128)
                        vt_ps = psA.tile([128, 128], f16, tag=f"o{h}",
                                         name=f"vtp_{h}_{nt}")
                        nc.tensor.transpose(vt_ps[:], vT_all[:, blk],
                                            ident16[:])
                        if VT_ACT:
                            nc.scalar.copy(v16_all[:, blk], vt_ps[:])
                        else:
                            nc.vector.tensor_copy(v16_all[:, blk], vt_ps[:])

            # ---- attention helpers ----
            def scores_tile(qc, kt, h, bt):
                qs0 = qc * QCW
                ks = slice(h * NK + kt * 128, h * NK + (kt + 1) * 128)
                qsl = slice(h * NQ + qs0, h * NQ + qs0 + QCW)
                sb = psA.tile([128, 1024], f32, tag="sb", bufs=2,
                              name=f"sb_{qc}_{kt}_{h}")
                nc.tensor.matmul(sb[:, 0:512], A_all[:, ks],
                                 q_all[:, qsl], start=True, stop=True)
                nc.tensor.matmul(sb[:, 512:1024], A_all[:, ks],
                                 q2_all[:, qsl], start=True, stop=True)
                # square extraction: sq = sb*sb (re^2 | im^2), f16
                sq = work.tile([128, 1024], f16, tag="sq", bufs=6,
                               name=f"sq_{qc}_{kt}_{h}")
                tile_in_half = (kt % HKT) * 2 + h
                if tile_in_half >= ACT_FUSED_FROM:
                    # unary ACT square reads psum once -- legal and fused
                    nc.scalar.square(sq[:], sb[:])
                elif tile_in_half >= MIXED_FROM:
                    # mixed: DVE does re-half copy+square, ACT squares im
                    t16 = work.tile([128, 1024], f16, tag="t16", bufs=6,
                                    name=f"t16_{qc}_{kt}_{h}")
                    nc.vector.tensor_copy(t16[:, 0:512], sb[:, 0:512])
                    nc.vector.tensor_tensor(sq[:, 0:512], t16[:, 0:512],
                                            t16[:, 0:512], ALU.mult)
                    nc.scalar.square(sq[:, 512:1024], sb[:, 512:1024])
                else:
                    # DVE path: one psum->sbuf copy, then sbuf squares
                    t16 = work.tile([128, 1024], f16, tag="t16", bufs=6,
                                    name=f"t16_{qc}_{kt}_{h}")
                    nc.vector.tensor_copy(t16[:], sb[:])
                    nc.vector.tensor_tensor(sq[:, 0:512], t16[:, 0:512],
                                            t16[:, 0:512], ALU.mult)
                    nc.gpsimd.tensor_tensor(sq[:, 512:1024],
                                            t16[:, 512:1024],
                                            t16[:, 512:1024], ALU.mult)
                cnt["ext"] += 1
                # z = re^2 + im^2 -> bt column slot
                zdst = bt[:, kt * 1024 + h * 512:kt * 1024 + h * 512 + 512]
                if ZADD_DVE_NUM and cnt["zadd"] % ZADD_MOD < ZADD_DVE_NUM:
                    nc.vector.tensor_tensor(zdst, sq[:, 0:512],
                                            sq[:, 512:1024], ALU.add)
                else:
                    nc.gpsimd.tensor_tensor(zdst, sq[:, 0:512],
                                            sq[:, 512:1024], ALU.add)
                cnt["zadd"] += 1

            def av_alloc(qc, last):
                o_ps = [psA.tile([128, QCW], f32, tag=f"o{h}",
                                 name=f"o{h}_{qc}") for h in (0, 1)]
                if last:
                    rs = psA.tile([128, 1024], f32, tag="aux",
                                  name=f"auxrs_{qc}")
                else:
                    rs = work.tile([128, 1024], f16, tag="rs_acc",
                                   bufs=2, name=f"rsacc_{qc}")
                return o_ps, rs

            def av_tile(qc, bt, kt, o_ps, rs, last):
                for h in (0, 1):
                    vblk = v16_all[:, h * NK + kt * 128:
                                   h * NK + (kt + 1) * 128]
                    pcol = bt[:, kt * 1024 + h * 512:
                              kt * 1024 + h * 512 + 512]
                    nc.tensor.matmul(o_ps[h][:, :], vblk, pcol,
                                     start=(kt == 0), stop=(kt == KT - 1))
                pk = bt[:, kt * 1024:(kt + 1) * 1024]
                if last:
                    # rowsum via PE directly into aux psum row 0
                    # (split 2x512 so each matmul stays within one bank)
                    for cb in (0, 1):
                        nc.tensor.matmul(
                            rs[0:1, cb * 512:(cb + 1) * 512], ones16[:],
                            pk[:, cb * 512:(cb + 1) * 512],
                            start=(kt == 0), stop=(kt == KT - 1))
                else:
                    # rowsum accumulation on Pool (both heads at once)
                    if kt == 0:
                        nc.gpsimd.tensor_scalar_mul(rs[:], pk, 1.0)
                    else:
                        nc.gpsimd.tensor_tensor(rs[:], rs[:], pk, ALU.add)

            def tail_pre(qc, o_ps, rs, last):
                qs = slice(qc * QCW, qc * QCW + QCW)
                if last:
                    aux = rs
                else:
                    aux = psA.tile([128, 1024], f32, tag="aux",
                                   name=f"aux_{qc}")
                    for cb in (0, 1):
                        nc.tensor.matmul(aux[0:1, cb * 512:(cb + 1) * 512],
                                         ones16[:],
                                         rs[:, cb * 512:(cb + 1) * 512],
                                         start=True, stop=True)
                rcp = work.tile([1, 1024], f32r, tag="rcp",
                                name=f"rcp_{qc}")
                nc.vector.reciprocal(rcp[:], aux[0:1, :])
                bc = psA.tile([128, 1024], f32, tag="aux",
                              name=f"bc_{qc}")
                for cb in (0, 1):
                    nc.tensor.matmul(bc[:, cb * 512:(cb + 1) * 512],
                                     ones_row[:],
                                     rcp[:, cb * 512:(cb + 1) * 512],
                                     start=True, stop=True)
                bc_sb = work.tile([128, 1024], f32r, tag="bc_sb",
                                  name=f"bcsb_{qc}")
                nc.scalar.copy(bc_sb[:], bc[:])
                # normalize AV output into oT (psum -> f16 sbuf)
                for h in (0, 1):
                    for ri, dest in ((0, oT_re), (1, oT_im)):
                        rows = slice(64 * ri, 64 * ri + 64)
                        nc.vector.scalar_tensor_tensor(
                            dest[64 * h:64 * h + 64, qs],
                            o_ps[h][rows, :], 1.0,
                            bc_sb[rows, h * 512:h * 512 + 512],
                            ALU.mult, ALU.mult)

            def tail_post(qc):
                qs = slice(qc * QCW, qc * QCW + QCW)
                for Rc in range(4):
                    def wob(w):
                        return wop[:, w * 512 + Rc * 128:
                                   w * 512 + Rc * 128 + 128]

                    for ri, (wa, wb_, dst_e) in enumerate(
                            ((0, 2, ore_e), (1, 0, oim_e))):
                        wo = psA.tile([128, 512], f32, tag="sb", bufs=2,
                                      name=f"wo_{Rc}_{ri}_{qc}")
                        nc.tensor.matmul(wo[:], wob(wa), oT_re[:, qs],
                                         start=True, stop=False)
                        nc.tensor.matmul(wo[:], wob(wb_), oT_im[:, qs],
                                         start=False, stop=True)
                        st = work.tile([128, 512], f32, tag="st", bufs=6,
                                       name=f"st_{Rc}_{ri}_{qc}")
                        if qc == QC - 1:
                            act_st = cnt["stc"] % 2 == 0  # parallel tail
                        else:
                            act_st = cnt["stc"] % ST_MOD < ST_ACT_NUM
                        if act_st:
                            nc.scalar.copy(st[:], wo[:])
                        else:
                            nc.vector.tensor_copy(st[:], wo[:])
                        cnt["stc"] += 1
                        nc.sync.dma_start(
                            dst_e[Rc * 128:(Rc + 1) * 128, qs], st[:])

            # ---- main schedule ----
            # Per q-chunk period: [A: AV+rowsum of qc-1 interleaved with
            # scores(qc, half0); sqrt(h0)] [B: tail of qc-1; scores(qc,
            # half1); sqrt(h1); exp(h0); exp(h1)]. ACT sees Square-only
            # between the two sqrts and the two exps -> 2 table loads/qc.
            proj(0)
            proj(1)
            pend = None
            for qc in range(QC):
                bt = work.tile([128, KT * 1024], f16, tag="batch",
                               bufs=2, name=f"bt_{qc}")
                if pend is not None:
                    pqc, pbt = pend
                    o_ps, rs = av_alloc(pqc, last=False)
                # A: scores(qc, h0-half k-tiles) + AV(pqc, first-half k-
                # tiles, gated on pqc's h0 exp which is already done)
                proj_a = {2: 2, 4: 3, 5: 4, 6: 5}   # k8 -> ncc (qc0 only)
                for k8 in range(HKT):
                    if qc == 0 and k8 in proj_a:
                        proj(proj_a[k8])
                    if pend is not None:
                        av_tile(pqc, pbt, k8, o_ps, rs, last=False)
                    scores_tile(qc, k8, 0, bt)
                    scores_tile(qc, k8, 1, bt)
                nc.scalar.activation(bt[:, 0:HKT * 1024],
                                     bt[:, 0:HKT * 1024], AF.Sqrt,
                                     scale=1.0 / 64.0)
                if qc == QC - 1:
                    # last q-chunk: exp(h0) early so its AV can overlap
                    nc.scalar.activation(bt[:, 0:HKT * 1024],
                                         bt[:, 0:HKT * 1024], AF.Exp,
                                         bias=eb_exp[:])
                # B: scores(qc, h1-half) + AV(pqc, second-half k-tiles)
                proj_b = {0: 6, 2: 7}               # k8 -> ncc (qc0 only)
                for k8 in range(HKT):
                    if qc == 0 and k8 in proj_b:
                        proj(proj_b[k8])
                    scores_tile(qc, HKT + k8, 0, bt)
                    scores_tile(qc, HKT + k8, 1, bt)
                    if pend is not None:
                        av_tile(pqc, pbt, HKT + k8, o_ps, rs, last=False)
                nc.scalar.activation(bt[:, HKT * 1024:],
                                     bt[:, HKT * 1024:], AF.Sqrt,
                                     scale=1.0 / 64.0)
                if pend is not None:
                    tail_pre(pqc, o_ps, rs, last=False)
                if qc != QC - 1:
                    for hh in range(4):
                        cs = slice(hh * 4096, (hh + 1) * 4096)
                        nc.scalar.activation(bt[:, cs], bt[:, cs], AF.Exp,
                                             bias=eb_exp[:])
                else:
                    # split the last exp so the final AV can start on the
                    # first quarter while the rest is still computing
                    for qtr in range(4):
                        cs = slice(HKT * 1024 + qtr * 2048,
                                   HKT * 1024 + (qtr + 1) * 2048)
                        nc.scalar.activation(bt[:, cs], bt[:, cs], AF.Exp,
                                             bias=eb_exp[:])
                if pend is not None:
                    tail_post(pqc)
                pend = (qc, bt)
            # final q-chunk: AV with PE rowsum (short tail), then tail
            pqc, pbt = pend
            o_ps, rs = av_alloc(pqc, last=True)
            for kt in range(KT):
                av_tile(pqc, pbt, kt, o_ps, rs, last=True)
            tail_pre(pqc, o_ps, rs, last=True)
            tail_post(pqc)

    nc.finalize()
    return nc


def _get_nc():
    if "nc" not in _CACHE:
        _CACHE["nc"] = _build_nc()
    return _CACHE["nc"]


def _core_inputs(c, inputs):
    b = c // 4
    h0 = 2 * (c % 4)
    hs = slice(h0 * 64, h0 * 64 + 128)

    xpack = np.empty((NCC, 24, 128, NCW), np.float16)
    for t, name in enumerate(
            ("Q_real", "Q_imag", "K_real", "K_imag", "V_real", "V_imag")):
        xT = np.ascontiguousarray(inputs[name][b].T)          # (512, 2048)
        xpack[:, t * 4:(t + 1) * 4] = (
            xT.reshape(4, 128, NCC, NCW).transpose(2, 0, 1, 3))

    wlist = []
    for kind in ("q", "A", "v"):
        base_r = inputs[{"q": "wq_r", "A": "wk_r", "v": "wv_r"}[kind]]
        base_i = inputs[{"q": "wq_i", "A": "wk_i", "v": "wv_i"}[kind]]
        for hh in (0, 1):
            rows = slice((h0 + hh) * 64, (h0 + hh) * 64 + 64)
            wr, wi_ = base_r[rows], base_i[rows]
            # rows of the projected tensor: [p_r; p_i]
            w1 = np.vstack([wr, wi_])        # x_re weights
            w2 = np.vstack([-wi_, wr])       # x_im weights
            wlist += [w1, w2]
    arr = np.empty((48, 128, 128), np.float16)
    for wi, mat in enumerate(wlist):
        arr[wi * 4:(wi + 1) * 4] = np.ascontiguousarray(mat.T).reshape(
            4, 128, 128)
    wpack = np.ascontiguousarray(arr.transpose(1, 0, 2)).reshape(
        128, 48 * 128)

    wo_r_T = np.ascontiguousarray(inputs["wo_r"][:, hs].T)    # (128, 512)
    wo_i_T = np.ascontiguousarray(inputs["wo_i"][:, hs].T)
    wopack = np.concatenate([wo_r_T, wo_i_T, -wo_i_T], axis=1)
    wopack = np.ascontiguousarray(wopack).astype(np.float16)

    return {
        "xpack": xpack,
        "wpack": wpack,
        "wopack": wopack,
        "onesr": np.ones((1, 128), np.float32),
        "ident": np.eye(128, dtype=np.float16),
    }


def kernel(**inputs):
    from concourse.bass_utils import run_bass_kernel_spmd

    nc = _get_nc()
    in_maps = [_core_inputs(c, inputs) for c in range(NCORES)]
    res = run_bass_kernel_spmd(nc, in_maps, list(range(NCORES)))
    out = np.empty((B, NQ, R, 2), np.float32)
    for b in range(B):
        re = np.zeros((512, NQ), np.float64)
        im = np.zeros((512, NQ), np.float64)
        for c in range(b * 4, b * 4 + 4):
            re += res.results[c]["out_re"]
            im += res.results[c]["out_im"]
        out[b, :, :, 0] = re.T
        out[b, :, :, 1] = im.T
    return out
